# revision 9
# baseline (speedup 1.0000x reference)
"""Trainium2 Bass kernel for a pre-LN transformer block (B=4,S=1024,E=1024,H=16).

Sharding: 8 cores = (batch b, parity p). Core (b,p) computes the full block for
batch b restricted to query tokens {q : q % 2 == p} (512 tokens). K/V are
computed for all 1024 tokens of the batch (duplicated across the pair), which
makes every core's instruction stream identical (pure SPMD) with per-core
differences carried entirely by input data (x slices + boundary masks).

Local token order: tile t in 0..3 holds global queries 256*(3-t)+2j+p, so the
causal context per tile is 1024-256t and AV matmul N shrinks monotonically.
"""

import math
import os

import numpy as np

B, S, E, H = 4, 1024, 1024, 16
DH = E // H
FF = 4 * E
P = 128
NCORES = 8
LN_EPS = 1e-5
NEG = -30000.0

_BUILD_CACHE = {}
LAST_RESULTS = None


def _build():
    if "nc" in _BUILD_CACHE:
        return _BUILD_CACHE["nc"]

    import concourse.bacc as bacc
    import concourse.bass as bass
    import concourse.mybir as mybir
    import concourse.tile as tile

    f32 = mybir.dt.float32
    bf16 = mybir.dt.bfloat16
    AF = mybir.ActivationFunctionType
    ALU = mybir.AluOpType

    nc = bacc.Bacc(
        "TRN2",
        target_bir_lowering=False,
        debug=False,
        enable_asserts=False,
        num_devices=NCORES,
    )

    # ---- DRAM I/O ----
    xg_d = nc.dram_tensor("x_glob", [S, E], f32, kind="ExternalInput")
    xl_d = nc.dram_tensor("x_loc", [S // 2, E], f32, kind="ExternalInput")
    wq_d = nc.dram_tensor("wq_sb", [P, 8, E], bf16, kind="ExternalInput")
    wk_d = nc.dram_tensor("wk_sb", [P, 8, E], bf16, kind="ExternalInput")
    wv_d = nc.dram_tensor("wv_sb", [P, 8, E], bf16, kind="ExternalInput")
    wo_d = nc.dram_tensor("wo_sb", [P, 8, E], bf16, kind="ExternalInput")
    wf_d = nc.dram_tensor("wf_sb", [8, P, 4, 8, P], bf16, kind="ExternalInput")
    wpa_d = nc.dram_tensor("wpa_sb", [8, P, 4, 512], bf16, kind="ExternalInput")
    wpb_d = nc.dram_tensor("wpb_sb", [8, P, 4, 512], bf16, kind="ExternalInput")
    bq_d = nc.dram_tensor("bq_sb", [P, 8], f32, kind="ExternalInput")
    bk_d = nc.dram_tensor("bk_sb", [P, 8], f32, kind="ExternalInput")
    bfc_d = nc.dram_tensor("bfc_sb", [P, 32], f32, kind="ExternalInput")
    bvec_d = nc.dram_tensor("bvec", [E], f32, kind="ExternalInput")
    bvec2_d = nc.dram_tensor("bvec2", [E], f32, kind="ExternalInput")
    msk_d = nc.dram_tensor("msk", [P, 4, 256], f32, kind="ExternalInput")
    yout_d = nc.dram_tensor("y_out", [S // 2, E], f32, kind="ExternalOutput")
    awout_d = nc.dram_tensor("aw_out", [S // 2, E], f32, kind="ExternalOutput")

    xg = xg_d.ap()
    xl = xl_d.ap()
    yout = yout_d.ap()
    awout = awout_d.ap()

    def bcast_ap(handle):
        ap = handle.ap()
        return bass.AP(tensor=ap.tensor, offset=ap.offset, ap=[[0, P]] + list(ap.ap))

    with tile.TileContext(nc) as tc:
        import contextlib

        stack = contextlib.ExitStack()
        with stack:
            persist = stack.enter_context(tc.tile_pool(name="persist", bufs=1))

            xres = persist.tile([P, 4, E], f32, tag="xres")
            ysb = persist.tile([P, 4, E], f32, tag="ysb")
            awsb = persist.tile([P, 4, S], f32, tag="awsb")
            awv = persist.tile([P, 4, S], bf16, tag="awv")
            z2T = persist.tile([P, 8, 512], bf16, tag="z2T")
            bvecb = persist.tile([P, E], f32, tag="bvecb")
            bvec2b = persist.tile([P, E], f32, tag="bvec2b")
            mskb = persist.tile([P, 4, 256], f32, tag="mskb")
            bqsb = persist.tile([P, 8], f32, tag="bqsb")
            bksb = persist.tile([P, 8], f32, tag="bksb")
            bfcsb = persist.tile([P, 32], f32, tag="bfcsb")
            epsb = persist.tile([P, 1], f32, tag="epsb")

            # constants
            nc.vector.memset(epsb[:], LN_EPS)
            nc.sync.dma_start(out=bqsb[:], in_=bq_d.ap())
            nc.sync.dma_start(out=bksb[:], in_=bk_d.ap())
            nc.sync.dma_start(out=bfcsb[:], in_=bfc_d.ap())
            nc.sync.dma_start(out=mskb[:], in_=msk_d.ap())
            nc.gpsimd.dma_start(out=bvecb[:], in_=bcast_ap(bvec_d))
            nc.gpsimd.dma_start(out=bvec2b[:], in_=bcast_ap(bvec2_d))
            nc.vector.memset(awsb[:], 0.0)
            nc.vector.memset(awv[:], 0.0)

            # attention-scope persistent tensors (freed before MLP)
            import contextlib as _ctl
            astack = _ctl.ExitStack()
            apool = astack.enter_context(tc.tile_pool(name="attnbuf", bufs=1))
            hstack = _ctl.ExitStack()
            hpool = hstack.enter_context(tc.tile_pool(name="hbuf", bufs=1))
            hT = hpool.tile([P, 8, S], bf16, tag="hT")
            hqT = apool.tile([P, 8, 512], bf16, tag="hqT")
            kdt = apool.tile([P, 8, S], bf16, tag="kdt")
            vtd = apool.tile([P, 8, E], bf16, tag="vtd")
            qdt = apool.tile([P, 8, 512], bf16, tag="qdt")
            attsb = apool.tile([P, 8, 512], bf16, tag="attsb")

            # ---------- Phase A: LayerNorm1 + transpose ----------
            def ln_tile(pool, spool, src_ap, out_bf):
                xt = pool.tile([P, E], f32, tag="ln_x")
                nc.gpsimd.dma_start(out=xt[:], in_=src_ap)
                st = spool.tile([P, 2, 6], f32, tag="ln_st")
                nc.vector.bn_stats(out=st[:, 0, :], in_=xt[:, 0:512])
                nc.vector.bn_stats(out=st[:, 1, :], in_=xt[:, 512:1024])
                mv = spool.tile([P, 2], f32, tag="ln_mv")
                nc.vector.bn_aggr(out=mv[:], in_=st[:])
                sd = spool.tile([P, 1], f32, tag="ln_sd")
                nc.scalar.activation(sd[:], mv[:, 1:2], AF.Sqrt, bias=epsb[:], scale=1.0)
                rs = spool.tile([P, 1], f32, tag="ln_rs")
                nc.vector.reciprocal(rs[:], sd[:])
                nmu = spool.tile([P, 1], f32, tag="ln_nmu")
                nc.vector.tensor_scalar(
                    out=nmu[:], in0=mv[:, 0:1], scalar1=rs[:], scalar2=-1.0,
                    op0=ALU.mult, op1=ALU.mult,
                )
                nc.scalar.activation(out_bf[:], xt[:], AF.Identity, bias=nmu[:], scale=rs[:])

            with tc.tile_pool(name="lnp", bufs=3) as lnp, tc.tile_pool(
                name="lns", bufs=8
            ) as lns:
                for tl in range(4):
                    zt = lnp.tile([P, E], bf16, tag="ln_z")
                    ln_tile(lnp, lns, xl[P * tl : P * (tl + 1), :], zt)
                    nc.sync.dma_start_transpose(
                        out=hqT[:, :, P * tl : P * (tl + 1)], in_=zt[:]
                    )
                for tg in range(8):
                    zt = lnp.tile([P, E], bf16, tag="ln_z")
                    ln_tile(lnp, lns, xg[P * tg : P * (tg + 1), :], zt)
                    nc.sync.dma_start_transpose(
                        out=hT[:, :, P * tg : P * (tg + 1)], in_=zt[:]
                    )
                for m in range(4):
                    nc.sync.dma_start(
                        out=xres[:, m, :], in_=xl[P * m : P * (m + 1), :]
                    )
                    nc.vector.tensor_add(xres[:, m, :], xres[:, m, :], bvecb[:])

            # ---------- Phase B: QKV projections ----------
            with tc.tile_pool(name="wqkv", bufs=2) as wp, tc.tile_pool(
                name="psqkv", bufs=4, space="PSUM"
            ) as pp:
                wq_sb = wp.tile([P, 8, E], bf16, tag="w")
                nc.scalar.dma_start(out=wq_sb[:], in_=wq_d.ap())
                for oc in range(8):
                    ps = pp.tile([P, 512], f32, tag="ps")
                    for kc in range(8):
                        nc.tensor.matmul(
                            ps[:],
                            lhsT=wq_sb[:, kc, P * oc : P * (oc + 1)],
                            rhs=hqT[:, kc, :],
                            start=(kc == 0),
                            stop=(kc == 7),
                        )
                    nc.scalar.activation(
                        qdt[:, oc, :],
                        ps[:],
                        AF.Identity,
                        bias=bqsb[:, oc : oc + 1],
                        scale=1.0,
                    )
                wk_sb = wp.tile([P, 8, E], bf16, tag="w")
                nc.scalar.dma_start(out=wk_sb[:], in_=wk_d.ap())
                for oc in range(8):
                    for ch in range(2):
                        ps = pp.tile([P, 512], f32, tag="ps")
                        for kc in range(8):
                            nc.tensor.matmul(
                                ps[:],
                                lhsT=wk_sb[:, kc, P * oc : P * (oc + 1)],
                                rhs=hT[:, kc, 512 * ch : 512 * (ch + 1)],
                                start=(kc == 0),
                                stop=(kc == 7),
                            )
                        nc.scalar.activation(
                            kdt[:, oc, 512 * ch : 512 * (ch + 1)],
                            ps[:],
                            AF.Identity,
                            bias=bksb[:, oc : oc + 1],
                            scale=1.0,
                        )
                wv_sb = wp.tile([P, 8, E], bf16, tag="w")
                nc.scalar.dma_start(out=wv_sb[:], in_=wv_d.ap())
                for tkt in range(8):
                    for ch in range(2):
                        ps = pp.tile([P, 512], f32, tag="ps")
                        for kc in range(8):
                            nc.tensor.matmul(
                                ps[:],
                                lhsT=hT[:, kc, P * tkt : P * (tkt + 1)],
                                rhs=wv_sb[:, kc, 512 * ch : 512 * (ch + 1)],
                                start=(kc == 0),
                                stop=(kc == 7),
                            )
                        nc.vector.tensor_copy(
                            vtd[:, tkt, 512 * ch : 512 * (ch + 1)], ps[:]
                        )

            hstack.close()

            # ---------- Phase C: attention ----------
            NK = [512, 512, 384, 384, 256, 256, 128, 128]
            with tc.tile_pool(name="wo", bufs=1) as wop:
                wo_sb = wop.tile([P, 8, E], bf16, tag="wo")
                nc.scalar.dma_start(out=wo_sb[:], in_=wo_d.ap())
                with tc.tile_pool(name="prb", bufs=2) as prp, tc.tile_pool(
                    name="psc", bufs=3, space="PSUM"
                ) as scp, tc.tile_pool(
                    name="psav", bufs=2, space="PSUM"
                ) as avp, tc.tile_pool(name="asm", bufs=8) as smp:

                    def emit_av(i, pTs):
                        avps = avp.tile([P, 512], f32, tag="av", name=f"av{i}")
                        for kc in range(8):
                            nk = NK[kc]
                            for sub in range(2):
                                h = 2 * i + sub
                                nc.tensor.matmul(
                                    avps[64 * sub : 64 * (sub + 1), :nk],
                                    lhsT=vtd[:, kc, 64 * h : 64 * (h + 1)],
                                    rhs=pTs[sub][:, kc, :nk],
                                    start=(kc == 0),
                                    stop=(kc == 7),
                                    skip_group_check=True,
                                )
                        nc.vector.tensor_copy(attsb[:, i, :], avps[:])

                    pend = None
                    for i in range(8):
                        pTs = [
                            prp.tile(
                                [P, 8, 512], bf16, tag="pT", name=f"pT{i}_{s}", bufs=4
                            )
                            for s in range(2)
                        ]
                        for tl in range(4):
                            ctx = S - 256 * tl
                            sc2 = [
                                scp.tile([P, S], f32, tag="sc", name=f"sc{i}_{tl}_{s}")
                                for s in range(2)
                            ]
                            for c0 in range(0, ctx, 512):
                                n = min(512, ctx - c0)
                                for sub in range(2):
                                    nc.tensor.matmul(
                                        sc2[sub][:, c0 : c0 + n],
                                        lhsT=qdt[
                                            64 * sub : 64 * (sub + 1),
                                            i,
                                            P * tl : P * (tl + 1),
                                        ],
                                        rhs=kdt[
                                            64 * sub : 64 * (sub + 1), i, c0 : c0 + n
                                        ],
                                        start=True,
                                        stop=True,
                                    )
                            for sub in range(2):
                                sc = sc2[sub]
                                nc.vector.tensor_add(
                                    sc[:, ctx - 256 : ctx],
                                    sc[:, ctx - 256 : ctx],
                                    mskb[:, tl, :],
                                )
                                pf = prp.tile([P, S], f32, tag="pf")
                                den = smp.tile([P, 1], f32, tag="den")
                                nc.scalar.activation(
                                    pf[:, :ctx],
                                    sc[:, :ctx],
                                    AF.Exp,
                                    accum_out=den[:],
                                )
                                r = smp.tile([P, 1], f32, tag="r")
                                nc.vector.reciprocal(r[:], den[:])
                                pbt = prp.tile([P, S], bf16, tag="pb", bufs=3)
                                if sub == 0:
                                    nc.gpsimd.tensor_scalar(
                                        out=pbt[:, :ctx],
                                        in0=pf[:, :ctx],
                                        scalar1=r[:],
                                        scalar2=None,
                                        op0=ALU.mult,
                                    )
                                    nc.gpsimd.tensor_tensor(
                                        awsb[:, tl, :ctx],
                                        awsb[:, tl, :ctx],
                                        pbt[:, :ctx],
                                        op=ALU.add,
                                    )
                                else:
                                    nc.scalar.activation(
                                        pbt[:, :ctx],
                                        pf[:, :ctx],
                                        AF.Copy,
                                        scale=r[:],
                                    )
                                    nc.vector.tensor_tensor(
                                        awv[:, tl, :ctx],
                                        awv[:, tl, :ctx],
                                        pbt[:, :ctx],
                                        op=ALU.add,
                                    )
                                nc.sync.dma_start_transpose(
                                    out=pTs[sub][:, : ctx // P, P * tl : P * (tl + 1)],
                                    in_=pbt[:, :ctx],
                                )
                        if pend is not None:
                            emit_av(*pend)
                        pend = (i, pTs)
                    emit_av(*pend)

                # ---------- Phase D: out_proj + residual ----------
                with tc.tile_pool(name="pso", bufs=2, space="PSUM") as pop:
                    for m in range(4):
                        for o2 in range(2):
                            ps = pop.tile([P, 512], f32, tag="po")
                            for i in range(8):
                                nc.tensor.matmul(
                                    ps[:],
                                    lhsT=attsb[:, i, P * m : P * (m + 1)],
                                    rhs=wo_sb[:, i, 512 * o2 : 512 * (o2 + 1)],
                                    start=(i == 0),
                                    stop=(i == 7),
                                )
                            nc.vector.tensor_add(
                                ysb[:, m, 512 * o2 : 512 * (o2 + 1)],
                                ps[:],
                                xres[:, m, 512 * o2 : 512 * (o2 + 1)],
                            )

            # ---------- Phase E: LayerNorm2 + transpose ----------
            with tc.tile_pool(name="ln2p", bufs=3) as lnp2, tc.tile_pool(
                name="ln2s", bufs=4
            ) as lns2:
                for m in range(4):
                    st = lns2.tile([P, 2, 6], f32, tag="l2st")
                    nc.vector.bn_stats(out=st[:, 0, :], in_=ysb[:, m, 0:512])
                    nc.vector.bn_stats(out=st[:, 1, :], in_=ysb[:, m, 512:1024])
                    mv = lns2.tile([P, 2], f32, tag="l2mv")
                    nc.vector.bn_aggr(out=mv[:], in_=st[:])
                    sd = lns2.tile([P, 1], f32, tag="l2sd")
                    nc.scalar.activation(
                        sd[:], mv[:, 1:2], AF.Sqrt, bias=epsb[:], scale=1.0
                    )
                    rs = lns2.tile([P, 1], f32, tag="l2rs")
                    nc.vector.reciprocal(rs[:], sd[:])
                    nmu = lns2.tile([P, 1], f32, tag="l2nmu")
                    nc.vector.tensor_scalar(
                        out=nmu[:], in0=mv[:, 0:1], scalar1=rs[:], scalar2=-1.0,
                        op0=ALU.mult, op1=ALU.mult,
                    )
                    z2 = lnp2.tile([P, E], bf16, tag="l2z")
                    nc.scalar.activation(
                        z2[:], ysb[:, m, :], AF.Identity, bias=nmu[:], scale=rs[:]
                    )
                    nc.sync.dma_start_transpose(
                        out=z2T[:, :, P * m : P * (m + 1)], in_=z2[:]
                    )

            # close attention buffers before MLP
            astack.close()

            # ---------- Phase F/G: MLP ----------
            with tc.tile_pool(name="mlp", bufs=1) as mp, tc.tile_pool(
                name="blk", bufs=2
            ) as bp, tc.tile_pool(name="x3p", bufs=3) as xp, tc.tile_pool(
                name="pspj", bufs=1, space="PSUM"
            ) as jp:
                hidT = mp.tile([P, 32, 512], bf16, tag="hidT")
                psA = [
                    jp.tile([P, 512], f32, tag=f"pja{m}", name=f"pja{m}")
                    for m in range(4)
                ]
                wfap = wf_d.ap()
                wpaap = wpa_d.ap()
                wpbap = wpb_d.ap()
                with tc.tile_pool(name="psf", bufs=2, space="PSUM") as fp:
                    for hg in range(8):
                        bf4 = bp.tile([P, 4, 8, P], bf16, tag="bf4")
                        nc.scalar.dma_start(out=bf4[:], in_=wfap[hg])
                        bpa4 = bp.tile([P, 4, 512], bf16, tag="bpa4")
                        nc.scalar.dma_start(out=bpa4[:], in_=wpaap[hg])
                        for hi in range(4):
                            hc = 4 * hg + hi
                            psF = fp.tile([P, 512], f32, tag="psF")
                            for kc in range(8):
                                nc.tensor.matmul(
                                    psF[:],
                                    lhsT=bf4[:, hi, kc, :],
                                    rhs=z2T[:, kc, :],
                                    start=(kc == 0),
                                    stop=(kc == 7),
                                )
                            nc.scalar.activation(
                                hidT[:, hc, :],
                                psF[:],
                                AF.Gelu_apprx_tanh,
                                bias=bfcsb[:, hc : hc + 1],
                                scale=1.0,
                            )
                            for m in range(4):
                                nc.tensor.matmul(
                                    psA[m][:],
                                    lhsT=hidT[:, hc, P * m : P * (m + 1)],
                                    rhs=bpa4[:, hi, :],
                                    start=(hc == 0),
                                    stop=(hc == 31),
                                )
                    for m in range(4):
                        x3 = xp.tile([P, 512], f32, tag="x3")
                        nc.vector.tensor_add(x3[:], psA[m][:], ysb[:, m, 0:512])
                        nc.vector.tensor_add(x3[:], x3[:], bvec2b[:, 0:512])
                        nc.sync.dma_start(
                            out=yout[P * m : P * (m + 1), 0:512], in_=x3[:]
                        )
                # att_weights finalize: merge halves, scale by 1/H, store
                for tl in range(4):
                    nc.vector.tensor_tensor(
                        awsb[:, tl, :], awsb[:, tl, :], awv[:, tl, :], op=ALU.add
                    )
                    nc.vector.tensor_scalar_mul(
                        awsb[:, tl, :], awsb[:, tl, :], 1.0 / H
                    )
                    nc.sync.dma_start(
                        out=awout[P * tl : P * (tl + 1), :], in_=awsb[:, tl, :]
                    )
                with tc.tile_pool(name="pspjb", bufs=1, space="PSUM") as jpb:
                    psB = [
                        jpb.tile([P, 512], f32, tag=f"pjb{m}", name=f"pjb{m}")
                        for m in range(4)
                    ]
                    for hg in range(8):
                        bpb4 = bp.tile([P, 4, 512], bf16, tag="bpb4")
                        nc.scalar.dma_start(out=bpb4[:], in_=wpbap[hg])
                        for hi in range(4):
                            hc = 4 * hg + hi
                            for m in range(4):
                                nc.tensor.matmul(
                                    psB[m][:],
                                    lhsT=hidT[:, hc, P * m : P * (m + 1)],
                                    rhs=bpb4[:, hi, :],
                                    start=(hc == 0),
                                    stop=(hc == 31),
                                )
                    for m in range(4):
                        x3 = xp.tile([P, 512], f32, tag="x3")
                        nc.vector.tensor_add(x3[:], psB[m][:], ysb[:, m, 512:1024])
                        nc.vector.tensor_add(x3[:], x3[:], bvec2b[:, 512:1024])
                        nc.sync.dma_start(
                            out=yout[P * m : P * (m + 1), 512:1024], in_=x3[:]
                        )

    nc.compile()
    _BUILD_CACHE["nc"] = nc
    return nc


def _local_rows(p):
    rows = []
    for t in range(4):
        g = 3 - t
        rows.extend(256 * g + 2 * j + p for j in range(P))
    return np.array(rows, dtype=np.int64)


def kernel(
    x,
    causal_mask,
    ln1_g,
    ln1_b,
    ln2_g,
    ln2_b,
    w_in,
    b_in,
    w_out,
    b_out,
    w_fc,
    b_fc,
    w_proj,
    b_proj,
):
    global LAST_RESULTS
    import ml_dtypes

    from concourse import bass_utils

    bf = ml_dtypes.bfloat16
    x = np.asarray(x, np.float32)
    causal_mask = np.asarray(causal_mask, np.float32)
    f32 = lambda a: np.ascontiguousarray(np.asarray(a, np.float32))
    ln1_g, ln1_b, ln2_g, ln2_b = map(f32, (ln1_g, ln1_b, ln2_g, ln2_b))
    w_in, b_in, w_out, b_out = map(f32, (w_in, b_in, w_out, b_out))
    w_fc, b_fc, w_proj, b_proj = map(f32, (w_fc, b_fc, w_proj, b_proj))

    wq, wk, wv = w_in[:E], w_in[E : 2 * E], w_in[2 * E :]
    bq, bk, bv = b_in[:E], b_in[E : 2 * E], b_in[2 * E :]
    scale = 1.0 / math.sqrt(DH)

    wq2 = (wq * ln1_g[None, :]) * scale
    bq2 = (wq @ ln1_b + bq) * scale
    wk2 = wk * ln1_g[None, :]
    bk2 = wk @ ln1_b + bk
    wv2 = wv * ln1_g[None, :]
    bv2 = wv @ ln1_b + bv
    bvec = b_out + w_out @ bv2
    wf2 = w_fc * ln2_g[None, :]
    bfc2 = b_fc + w_fc @ ln2_b
    bvec2 = b_proj

    def tile_w(wT2):
        # [out,in] weight -> SBUF layout [p, kc, o] with p the contraction row
        return np.ascontiguousarray(
            wT2.T.reshape(8, P, E).transpose(1, 0, 2).astype(bf)
        )

    wq_sb = tile_w(wq2)
    wk_sb = tile_w(wk2)
    wv_sb = tile_w(wv2)
    wo_sb = tile_w(w_out)
    # w_fc': [FF, E] -> [hg, p(k), hi, kc, c] so each hg slice is one DMA
    wf_sb = np.ascontiguousarray(
        wf2.reshape(8, 4, P, 8, P).transpose(0, 4, 1, 3, 2).astype(bf)
    )
    wpT = np.ascontiguousarray(w_proj.T)  # [FF, E]
    # [hg, p(h-row), hi, c]
    wpa_sb = np.ascontiguousarray(
        wpT[:, :512].reshape(8, 4, P, 512).transpose(0, 2, 1, 3).astype(bf)
    )
    wpb_sb = np.ascontiguousarray(
        wpT[:, 512:].reshape(8, 4, P, 512).transpose(0, 2, 1, 3).astype(bf)
    )
    bq_sb = np.ascontiguousarray(bq2.reshape(8, P).T)
    bk_sb = np.ascontiguousarray(bk2.reshape(8, P).T)
    bfc_sb = np.ascontiguousarray(bfc2.reshape(32, P).T)

    cm = np.maximum(causal_mask, NEG)

    in_maps = []
    rows_by_p = [_local_rows(0), _local_rows(1)]
    msk_by_p = []
    for p in range(2):
        msk = np.empty((P, 4, 256), np.float32)
        for t in range(4):
            ctx = S - 256 * t
            rows_t = rows_by_p[p][P * t : P * (t + 1)]
            msk[:, t, :] = cm[rows_t, ctx - 256 : ctx]
        msk_by_p.append(msk)

    shared = dict(
        wq_sb=wq_sb,
        wk_sb=wk_sb,
        wv_sb=wv_sb,
        wo_sb=wo_sb,
        wf_sb=wf_sb,
        wpa_sb=wpa_sb,
        wpb_sb=wpb_sb,
        bq_sb=bq_sb,
        bk_sb=bk_sb,
        bfc_sb=bfc_sb,
        bvec=np.ascontiguousarray(bvec),
        bvec2=np.ascontiguousarray(bvec2),
    )
    for c in range(NCORES):
        b, p = c // 2, c % 2
        m = dict(shared)
        m["x_glob"] = np.ascontiguousarray(x[b])
        m["x_loc"] = np.ascontiguousarray(x[b][rows_by_p[p]])
        m["msk"] = msk_by_p[p]
        in_maps.append(m)

    nc = _build()
    trace = bool(os.environ.get("KERNEL_TRACE"))
    res = bass_utils.run_bass_kernel_spmd(
        nc, in_maps, list(range(NCORES)), trace=trace
    )
    LAST_RESULTS = res

    x_out = np.empty((B, S, E), np.float32)
    att_w = np.empty((B, S, S), np.float32)
    for c in range(NCORES):
        b, p = c // 2, c % 2
        rows = rows_by_p[p]
        x_out[b][rows] = res.results[c]["y_out"]
        att_w[b][rows] = res.results[c]["aw_out"]
    return (x_out, att_w)


# revision 11
# speedup vs baseline: 1.4783x; 1.4783x over previous
"""Trainium2 Bass kernel for a pre-LN transformer block (B=4,S=1024,E=1024,H=16).

Sharding: 8 cores = (batch b, parity p). Core (b,p) computes the full block for
batch b restricted to query tokens {q : q % 2 == p} (512 tokens). K/V are
computed for all 1024 tokens of the batch (duplicated across the pair), which
makes every core's instruction stream identical (pure SPMD) with per-core
differences carried entirely by input data (x slices + boundary masks).

Local token order: tile t in 0..3 holds global queries 256*(3-t)+2j+p, so the
causal context per tile is 1024-256t and AV matmul N shrinks monotonically.
"""

import math
import os

import numpy as np

B, S, E, H = 4, 1024, 1024, 16
DH = E // H
FF = 4 * E
P = 128
NCORES = 8
LN_EPS = 1e-5
NEG = -30000.0

_BUILD_CACHE = {}
LAST_RESULTS = None


def _build():
    if "nc" in _BUILD_CACHE:
        return _BUILD_CACHE["nc"]

    import concourse.bacc as bacc
    import concourse.bass as bass
    import concourse.mybir as mybir
    import concourse.tile as tile

    f32 = mybir.dt.float32
    bf16 = mybir.dt.bfloat16
    AF = mybir.ActivationFunctionType
    ALU = mybir.AluOpType

    nc = bacc.Bacc(
        "TRN2",
        target_bir_lowering=False,
        debug=False,
        enable_asserts=False,
        num_devices=NCORES,
    )

    # ---- DRAM I/O ----
    xg_d = nc.dram_tensor("x_glob", [S, E], f32, kind="ExternalInput")
    xl_d = nc.dram_tensor("x_loc", [S // 2, E], f32, kind="ExternalInput")
    wq_d = nc.dram_tensor("wq_sb", [P, 8, E], bf16, kind="ExternalInput")
    wk_d = nc.dram_tensor("wk_sb", [P, 8, E], bf16, kind="ExternalInput")
    wv_d = nc.dram_tensor("wv_sb", [P, 8, E], bf16, kind="ExternalInput")
    wo_d = nc.dram_tensor("wo_sb", [P, 8, E], bf16, kind="ExternalInput")
    wf_d = nc.dram_tensor("wf_sb", [8, P, 4, 8, P], bf16, kind="ExternalInput")
    wpa_d = nc.dram_tensor("wpa_sb", [8, P, 4, 512], bf16, kind="ExternalInput")
    wpb_d = nc.dram_tensor("wpb_sb", [8, P, 4, 512], bf16, kind="ExternalInput")
    bq_d = nc.dram_tensor("bq_sb", [P, 8], f32, kind="ExternalInput")
    bk_d = nc.dram_tensor("bk_sb", [P, 8], f32, kind="ExternalInput")
    bfc_d = nc.dram_tensor("bfc_sb", [P, 32], f32, kind="ExternalInput")
    bvec_d = nc.dram_tensor("bvec", [E], f32, kind="ExternalInput")
    bvec2_d = nc.dram_tensor("bvec2", [E], f32, kind="ExternalInput")
    msk_d = nc.dram_tensor("msk", [P, 4, 256], f32, kind="ExternalInput")
    yout_d = nc.dram_tensor("y_out", [S // 2, E], f32, kind="ExternalOutput")
    awout_d = nc.dram_tensor("aw_out", [S // 2, E], f32, kind="ExternalOutput")

    xg = xg_d.ap()
    xl = xl_d.ap()
    yout = yout_d.ap()
    awout = awout_d.ap()

    def bcast_ap(handle):
        ap = handle.ap()
        return bass.AP(tensor=ap.tensor, offset=ap.offset, ap=[[0, P]] + list(ap.ap))

    with tile.TileContext(nc) as tc:
        import contextlib

        stack = contextlib.ExitStack()
        with stack:
            persist = stack.enter_context(tc.tile_pool(name="persist", bufs=1))

            xres = persist.tile([P, 4, E], f32, tag="xres")
            ysb = persist.tile([P, 4, E], f32, tag="ysb")
            awsb = persist.tile([P, 4, S], f32, tag="awsb")
            awv = persist.tile([P, 4, S], bf16, tag="awv")
            z2T = persist.tile([P, 8, 512], bf16, tag="z2T")
            bvecb = persist.tile([P, E], f32, tag="bvecb")
            bvec2b = persist.tile([P, E], f32, tag="bvec2b")
            mskb = persist.tile([P, 4, 256], f32, tag="mskb")
            bqsb = persist.tile([P, 8], f32, tag="bqsb")
            bksb = persist.tile([P, 8], f32, tag="bksb")
            bfcsb = persist.tile([P, 32], f32, tag="bfcsb")
            epsb = persist.tile([P, 1], f32, tag="epsb")

            # constants
            nc.vector.memset(epsb[:], LN_EPS)
            nc.sync.dma_start(out=bqsb[:], in_=bq_d.ap())
            nc.sync.dma_start(out=bksb[:], in_=bk_d.ap())
            nc.sync.dma_start(out=bfcsb[:], in_=bfc_d.ap())
            nc.sync.dma_start(out=mskb[:], in_=msk_d.ap())
            nc.gpsimd.dma_start(out=bvecb[:], in_=bcast_ap(bvec_d))
            nc.gpsimd.dma_start(out=bvec2b[:], in_=bcast_ap(bvec2_d))
            nc.vector.memset(awsb[:], 0.0)
            nc.vector.memset(awv[:], 0.0)

            # attention-scope persistent tensors (freed before MLP)
            import contextlib as _ctl
            astack = _ctl.ExitStack()
            apool = astack.enter_context(tc.tile_pool(name="attnbuf", bufs=1))
            hstack = _ctl.ExitStack()
            hpool = hstack.enter_context(tc.tile_pool(name="hbuf", bufs=1))
            hT = hpool.tile([P, 8, S], bf16, tag="hT")
            hqT = apool.tile([P, 8, 512], bf16, tag="hqT")
            kdt = apool.tile([P, 8, S], bf16, tag="kdt")
            vtd = apool.tile([P, 8, E], bf16, tag="vtd")
            qdt = apool.tile([P, 8, 512], bf16, tag="qdt")
            attsb = apool.tile([P, 8, 512], bf16, tag="attsb")

            # ---------- Phase A: LayerNorm1 (batched sweeps) + transpose ----------
            with tc.tile_pool(name="lnp", bufs=3) as lnp, tc.tile_pool(
                name="lnx", bufs=4
            ) as lnxp, tc.tile_pool(name="lns", bufs=2) as lns:

                _lnctr = [0]

                def ln_batch(srcs, dsts):
                    nb = len(srcs)
                    _lnctr[0] += 1
                    xts = []
                    mvb = lns.tile([P, nb, 2], f32, tag="ln_mv", name=f"mv{_lnctr[0]}")
                    for t in range(nb):
                        xt = lnxp.tile([P, E], f32, tag="ln_x")
                        nc.gpsimd.dma_start(out=xt[:], in_=srcs[t])
                        xts.append(xt)
                        st = lns.tile([P, 2, 6], f32, tag="ln_st", bufs=4)
                        nc.vector.bn_stats(out=st[:, 0, :], in_=xt[:, 0:512])
                        nc.vector.bn_stats(out=st[:, 1, :], in_=xt[:, 512:1024])
                        nc.vector.bn_aggr(out=mvb[:, t, :], in_=st[:])
                    sdb = lns.tile([P, nb], f32, tag="ln_sd")
                    nc.scalar.activation(
                        sdb[:], mvb[:, :, 1], AF.Sqrt, bias=epsb[:], scale=1.0
                    )
                    rsb = lns.tile([P, nb], f32, tag="ln_rs")
                    nc.vector.reciprocal(rsb[:], sdb[:])
                    nmub = lns.tile([P, nb], f32, tag="ln_nmu")
                    nc.vector.tensor_tensor(nmub[:], mvb[:, :, 0], rsb[:], op=ALU.mult)
                    nc.vector.tensor_scalar_mul(nmub[:], nmub[:], -1.0)
                    for t in range(nb):
                        zt = lnp.tile([P, E], bf16, tag="ln_z")
                        nc.scalar.activation(
                            zt[:],
                            xts[t][:],
                            AF.Identity,
                            bias=nmub[:, t : t + 1],
                            scale=rsb[:, t : t + 1],
                        )
                        nc.sync.dma_start_transpose(out=dsts[t], in_=zt[:])

                ln_batch(
                    [xl[P * t : P * (t + 1), :] for t in range(4)],
                    [hqT[:, :, P * t : P * (t + 1)] for t in range(4)],
                )
                ln_batch(
                    [xg[P * t : P * (t + 1), :] for t in range(4)],
                    [hT[:, :, P * t : P * (t + 1)] for t in range(4)],
                )
                ln_batch(
                    [xg[P * t : P * (t + 1), :] for t in range(4, 8)],
                    [hT[:, :, P * t : P * (t + 1)] for t in range(4, 8)],
                )
                for m in range(4):
                    nc.sync.dma_start(
                        out=xres[:, m, :], in_=xl[P * m : P * (m + 1), :]
                    )
                    nc.vector.tensor_add(xres[:, m, :], xres[:, m, :], bvecb[:])

            # ---------- Phase B: QKV projections ----------
            with tc.tile_pool(name="wqkv", bufs=2) as wp, tc.tile_pool(
                name="psqkv", bufs=4, space="PSUM"
            ) as pp:
                wq_sb = wp.tile([P, 8, E], bf16, tag="w")
                nc.scalar.dma_start(out=wq_sb[:], in_=wq_d.ap())
                for oc in range(8):
                    ps = pp.tile([P, 512], f32, tag="ps")
                    for kc in range(8):
                        nc.tensor.matmul(
                            ps[:],
                            lhsT=wq_sb[:, kc, P * oc : P * (oc + 1)],
                            rhs=hqT[:, kc, :],
                            start=(kc == 0),
                            stop=(kc == 7),
                        )
                    nc.scalar.activation(
                        qdt[:, oc, :],
                        ps[:],
                        AF.Identity,
                        bias=bqsb[:, oc : oc + 1],
                        scale=1.0,
                    )
                wk_sb = wp.tile([P, 8, E], bf16, tag="w")
                nc.scalar.dma_start(out=wk_sb[:], in_=wk_d.ap())
                for oc in range(8):
                    for ch in range(2):
                        ps = pp.tile([P, 512], f32, tag="ps")
                        for kc in range(8):
                            nc.tensor.matmul(
                                ps[:],
                                lhsT=wk_sb[:, kc, P * oc : P * (oc + 1)],
                                rhs=hT[:, kc, 512 * ch : 512 * (ch + 1)],
                                start=(kc == 0),
                                stop=(kc == 7),
                            )
                        nc.scalar.activation(
                            kdt[:, oc, 512 * ch : 512 * (ch + 1)],
                            ps[:],
                            AF.Identity,
                            bias=bksb[:, oc : oc + 1],
                            scale=1.0,
                        )
                wv_sb = wp.tile([P, 8, E], bf16, tag="w")
                nc.scalar.dma_start(out=wv_sb[:], in_=wv_d.ap())
                for tkt in range(8):
                    for ch in range(2):
                        ps = pp.tile([P, 512], f32, tag="ps")
                        for kc in range(8):
                            nc.tensor.matmul(
                                ps[:],
                                lhsT=hT[:, kc, P * tkt : P * (tkt + 1)],
                                rhs=wv_sb[:, kc, 512 * ch : 512 * (ch + 1)],
                                start=(kc == 0),
                                stop=(kc == 7),
                            )
                        nc.vector.tensor_copy(
                            vtd[:, tkt, 512 * ch : 512 * (ch + 1)], ps[:]
                        )

            hstack.close()

            # ---------- Phase C: attention ----------
            NK = [512, 512, 384, 384, 256, 256, 128, 128]
            with tc.tile_pool(name="wo", bufs=1) as wop:
                wo_sb = wop.tile([P, 8, E], bf16, tag="wo")
                nc.scalar.dma_start(out=wo_sb[:], in_=wo_d.ap())
                with tc.tile_pool(name="prb", bufs=2) as prp, tc.tile_pool(
                    name="psc", bufs=3, space="PSUM"
                ) as scp, tc.tile_pool(
                    name="psav", bufs=2, space="PSUM"
                ) as avp, tc.tile_pool(name="asm", bufs=8) as smp:

                    def emit_av(i, pTs):
                        avps = avp.tile([P, 512], f32, tag="av", name=f"av{i}")
                        for kc in range(8):
                            nk = NK[kc]
                            for sub in range(2):
                                h = 2 * i + sub
                                nc.tensor.matmul(
                                    avps[64 * sub : 64 * (sub + 1), :nk],
                                    lhsT=vtd[:, kc, 64 * h : 64 * (h + 1)],
                                    rhs=pTs[sub][:, kc, :nk],
                                    start=(kc == 0),
                                    stop=(kc == 7),
                                    skip_group_check=True,
                                )
                        nc.vector.tensor_copy(attsb[:, i, :], avps[:])

                    pend = None
                    for i in range(8):
                        pTs = [
                            prp.tile(
                                [P, 8, 512], bf16, tag="pT", name=f"pT{i}_{s}", bufs=4
                            )
                            for s in range(2)
                        ]
                        for tl in range(4):
                            ctx = S - 256 * tl
                            sc2 = [
                                scp.tile([P, S], f32, tag="sc", name=f"sc{i}_{tl}_{s}")
                                for s in range(2)
                            ]
                            for c0 in range(0, ctx, 512):
                                n = min(512, ctx - c0)
                                for sub in range(2):
                                    nc.tensor.matmul(
                                        sc2[sub][:, c0 : c0 + n],
                                        lhsT=qdt[
                                            64 * sub : 64 * (sub + 1),
                                            i,
                                            P * tl : P * (tl + 1),
                                        ],
                                        rhs=kdt[
                                            64 * sub : 64 * (sub + 1), i, c0 : c0 + n
                                        ],
                                        start=True,
                                        stop=True,
                                    )
                            for sub in range(2):
                                sc = sc2[sub]
                                nc.vector.tensor_add(
                                    sc[:, ctx - 256 : ctx],
                                    sc[:, ctx - 256 : ctx],
                                    mskb[:, tl, :],
                                )
                                pf = prp.tile([P, S], f32, tag="pf")
                                den = smp.tile([P, 1], f32, tag="den")
                                nc.scalar.activation(
                                    pf[:, :ctx],
                                    sc[:, :ctx],
                                    AF.Exp,
                                    accum_out=den[:],
                                )
                                r = smp.tile([P, 1], f32, tag="r")
                                nc.vector.reciprocal(r[:], den[:])
                                pbt = prp.tile([P, S], bf16, tag="pb", bufs=3)
                                if sub == 0:
                                    nc.vector.tensor_scalar(
                                        out=pbt[:, :ctx],
                                        in0=pf[:, :ctx],
                                        scalar1=r[:],
                                        scalar2=None,
                                        op0=ALU.mult,
                                    )
                                    nc.gpsimd.tensor_tensor(
                                        awsb[:, tl, :ctx],
                                        awsb[:, tl, :ctx],
                                        pbt[:, :ctx],
                                        op=ALU.add,
                                    )
                                else:
                                    nc.scalar.activation(
                                        pbt[:, :ctx],
                                        pf[:, :ctx],
                                        AF.Copy,
                                        scale=r[:],
                                    )
                                    nc.vector.tensor_tensor(
                                        awv[:, tl, :ctx],
                                        awv[:, tl, :ctx],
                                        pbt[:, :ctx],
                                        op=ALU.add,
                                    )
                                nc.sync.dma_start_transpose(
                                    out=pTs[sub][:, : ctx // P, P * tl : P * (tl + 1)],
                                    in_=pbt[:, :ctx],
                                )
                        if pend is not None:
                            emit_av(*pend)
                        pend = (i, pTs)
                    emit_av(*pend)

                # ---------- Phase D: out_proj + residual ----------
                with tc.tile_pool(name="pso", bufs=2, space="PSUM") as pop:
                    for m in range(4):
                        for o2 in range(2):
                            ps = pop.tile([P, 512], f32, tag="po")
                            for i in range(8):
                                nc.tensor.matmul(
                                    ps[:],
                                    lhsT=attsb[:, i, P * m : P * (m + 1)],
                                    rhs=wo_sb[:, i, 512 * o2 : 512 * (o2 + 1)],
                                    start=(i == 0),
                                    stop=(i == 7),
                                )
                            nc.vector.tensor_add(
                                ysb[:, m, 512 * o2 : 512 * (o2 + 1)],
                                ps[:],
                                xres[:, m, 512 * o2 : 512 * (o2 + 1)],
                            )

            # ---------- Phase E: LayerNorm2 + transpose ----------
            with tc.tile_pool(name="ln2p", bufs=3) as lnp2, tc.tile_pool(
                name="ln2s", bufs=4
            ) as lns2:
                for m in range(4):
                    st = lns2.tile([P, 2, 6], f32, tag="l2st")
                    nc.vector.bn_stats(out=st[:, 0, :], in_=ysb[:, m, 0:512])
                    nc.vector.bn_stats(out=st[:, 1, :], in_=ysb[:, m, 512:1024])
                    mv = lns2.tile([P, 2], f32, tag="l2mv")
                    nc.vector.bn_aggr(out=mv[:], in_=st[:])
                    sd = lns2.tile([P, 1], f32, tag="l2sd")
                    nc.scalar.activation(
                        sd[:], mv[:, 1:2], AF.Sqrt, bias=epsb[:], scale=1.0
                    )
                    rs = lns2.tile([P, 1], f32, tag="l2rs")
                    nc.vector.reciprocal(rs[:], sd[:])
                    nmu = lns2.tile([P, 1], f32, tag="l2nmu")
                    nc.vector.tensor_scalar(
                        out=nmu[:], in0=mv[:, 0:1], scalar1=rs[:], scalar2=-1.0,
                        op0=ALU.mult, op1=ALU.mult,
                    )
                    z2 = lnp2.tile([P, E], bf16, tag="l2z")
                    nc.scalar.activation(
                        z2[:], ysb[:, m, :], AF.Identity, bias=nmu[:], scale=rs[:]
                    )
                    nc.sync.dma_start_transpose(
                        out=z2T[:, :, P * m : P * (m + 1)], in_=z2[:]
                    )

            # close attention buffers before MLP
            astack.close()

            # ---------- Phase F/G: MLP ----------
            with tc.tile_pool(name="mlp", bufs=1) as mp, tc.tile_pool(
                name="blk", bufs=2
            ) as bp, tc.tile_pool(name="x3p", bufs=3) as xp, tc.tile_pool(
                name="pspj", bufs=1, space="PSUM"
            ) as jp:
                hidT = mp.tile([P, 32, 512], bf16, tag="hidT")
                psA = [
                    jp.tile([P, 512], f32, tag=f"pja{m}", name=f"pja{m}")
                    for m in range(4)
                ]
                wfap = wf_d.ap()
                wpaap = wpa_d.ap()
                wpbap = wpb_d.ap()
                with tc.tile_pool(name="psf", bufs=2, space="PSUM") as fp:
                    for hg in range(8):
                        bf4 = bp.tile([P, 4, 8, P], bf16, tag="bf4")
                        nc.scalar.dma_start(out=bf4[:], in_=wfap[hg])
                        bpa4 = bp.tile([P, 4, 512], bf16, tag="bpa4")
                        nc.scalar.dma_start(out=bpa4[:], in_=wpaap[hg])
                        for hi in range(4):
                            hc = 4 * hg + hi
                            psF = fp.tile([P, 512], f32, tag="psF")
                            for kc in range(8):
                                nc.tensor.matmul(
                                    psF[:],
                                    lhsT=bf4[:, hi, kc, :],
                                    rhs=z2T[:, kc, :],
                                    start=(kc == 0),
                                    stop=(kc == 7),
                                )
                            nc.scalar.activation(
                                hidT[:, hc, :],
                                psF[:],
                                AF.Gelu_apprx_tanh,
                                bias=bfcsb[:, hc : hc + 1],
                                scale=1.0,
                            )
                            for m in range(4):
                                nc.tensor.matmul(
                                    psA[m][:],
                                    lhsT=hidT[:, hc, P * m : P * (m + 1)],
                                    rhs=bpa4[:, hi, :],
                                    start=(hc == 0),
                                    stop=(hc == 31),
                                )
                    for m in range(4):
                        x3 = xp.tile([P, 512], f32, tag="x3")
                        nc.vector.tensor_add(x3[:], psA[m][:], ysb[:, m, 0:512])
                        nc.vector.tensor_add(x3[:], x3[:], bvec2b[:, 0:512])
                        nc.sync.dma_start(
                            out=yout[P * m : P * (m + 1), 0:512], in_=x3[:]
                        )
                # att_weights finalize: merge halves, scale by 1/H, store
                for tl in range(4):
                    nc.vector.tensor_tensor(
                        awsb[:, tl, :], awsb[:, tl, :], awv[:, tl, :], op=ALU.add
                    )
                    nc.vector.tensor_scalar_mul(
                        awsb[:, tl, :], awsb[:, tl, :], 1.0 / H
                    )
                    nc.sync.dma_start(
                        out=awout[P * tl : P * (tl + 1), :], in_=awsb[:, tl, :]
                    )
                with tc.tile_pool(name="pspjb", bufs=1, space="PSUM") as jpb:
                    psB = [
                        jpb.tile([P, 512], f32, tag=f"pjb{m}", name=f"pjb{m}")
                        for m in range(4)
                    ]
                    for hg in range(8):
                        bpb4 = bp.tile([P, 4, 512], bf16, tag="bpb4")
                        nc.scalar.dma_start(out=bpb4[:], in_=wpbap[hg])
                        for hi in range(4):
                            hc = 4 * hg + hi
                            for m in range(4):
                                nc.tensor.matmul(
                                    psB[m][:],
                                    lhsT=hidT[:, hc, P * m : P * (m + 1)],
                                    rhs=bpb4[:, hi, :],
                                    start=(hc == 0),
                                    stop=(hc == 31),
                                )
                    for m in range(4):
                        x3 = xp.tile([P, 512], f32, tag="x3")
                        nc.vector.tensor_add(x3[:], psB[m][:], ysb[:, m, 512:1024])
                        nc.vector.tensor_add(x3[:], x3[:], bvec2b[:, 512:1024])
                        nc.sync.dma_start(
                            out=yout[P * m : P * (m + 1), 512:1024], in_=x3[:]
                        )

    nc.compile()
    _BUILD_CACHE["nc"] = nc
    return nc


def _local_rows(p):
    rows = []
    for t in range(4):
        g = 3 - t
        rows.extend(256 * g + 2 * j + p for j in range(P))
    return np.array(rows, dtype=np.int64)


def kernel(
    x,
    causal_mask,
    ln1_g,
    ln1_b,
    ln2_g,
    ln2_b,
    w_in,
    b_in,
    w_out,
    b_out,
    w_fc,
    b_fc,
    w_proj,
    b_proj,
):
    global LAST_RESULTS
    import ml_dtypes

    from concourse import bass_utils

    bf = ml_dtypes.bfloat16
    x = np.asarray(x, np.float32)
    causal_mask = np.asarray(causal_mask, np.float32)
    f32 = lambda a: np.ascontiguousarray(np.asarray(a, np.float32))
    ln1_g, ln1_b, ln2_g, ln2_b = map(f32, (ln1_g, ln1_b, ln2_g, ln2_b))
    w_in, b_in, w_out, b_out = map(f32, (w_in, b_in, w_out, b_out))
    w_fc, b_fc, w_proj, b_proj = map(f32, (w_fc, b_fc, w_proj, b_proj))

    wq, wk, wv = w_in[:E], w_in[E : 2 * E], w_in[2 * E :]
    bq, bk, bv = b_in[:E], b_in[E : 2 * E], b_in[2 * E :]
    scale = 1.0 / math.sqrt(DH)

    wq2 = (wq * ln1_g[None, :]) * scale
    bq2 = (wq @ ln1_b + bq) * scale
    wk2 = wk * ln1_g[None, :]
    bk2 = wk @ ln1_b + bk
    wv2 = wv * ln1_g[None, :]
    bv2 = wv @ ln1_b + bv
    bvec = b_out + w_out @ bv2
    wf2 = w_fc * ln2_g[None, :]
    bfc2 = b_fc + w_fc @ ln2_b
    bvec2 = b_proj

    def tile_w(wT2):
        # [out,in] weight -> SBUF layout [p, kc, o] with p the contraction row
        return np.ascontiguousarray(
            wT2.T.reshape(8, P, E).transpose(1, 0, 2).astype(bf)
        )

    wq_sb = tile_w(wq2)
    wk_sb = tile_w(wk2)
    wv_sb = tile_w(wv2)
    wo_sb = tile_w(w_out)
    # w_fc': [FF, E] -> [hg, p(k), hi, kc, c] so each hg slice is one DMA
    wf_sb = np.ascontiguousarray(
        wf2.reshape(8, 4, P, 8, P).transpose(0, 4, 1, 3, 2).astype(bf)
    )
    wpT = np.ascontiguousarray(w_proj.T)  # [FF, E]
    # [hg, p(h-row), hi, c]
    wpa_sb = np.ascontiguousarray(
        wpT[:, :512].reshape(8, 4, P, 512).transpose(0, 2, 1, 3).astype(bf)
    )
    wpb_sb = np.ascontiguousarray(
        wpT[:, 512:].reshape(8, 4, P, 512).transpose(0, 2, 1, 3).astype(bf)
    )
    bq_sb = np.ascontiguousarray(bq2.reshape(8, P).T)
    bk_sb = np.ascontiguousarray(bk2.reshape(8, P).T)
    bfc_sb = np.ascontiguousarray(bfc2.reshape(32, P).T)

    cm = np.maximum(causal_mask, NEG)

    in_maps = []
    rows_by_p = [_local_rows(0), _local_rows(1)]
    msk_by_p = []
    for p in range(2):
        msk = np.empty((P, 4, 256), np.float32)
        for t in range(4):
            ctx = S - 256 * t
            rows_t = rows_by_p[p][P * t : P * (t + 1)]
            msk[:, t, :] = cm[rows_t, ctx - 256 : ctx]
        msk_by_p.append(msk)

    shared = dict(
        wq_sb=wq_sb,
        wk_sb=wk_sb,
        wv_sb=wv_sb,
        wo_sb=wo_sb,
        wf_sb=wf_sb,
        wpa_sb=wpa_sb,
        wpb_sb=wpb_sb,
        bq_sb=bq_sb,
        bk_sb=bk_sb,
        bfc_sb=bfc_sb,
        bvec=np.ascontiguousarray(bvec),
        bvec2=np.ascontiguousarray(bvec2),
    )
    for c in range(NCORES):
        b, p = c // 2, c % 2
        m = dict(shared)
        m["x_glob"] = np.ascontiguousarray(x[b])
        m["x_loc"] = np.ascontiguousarray(x[b][rows_by_p[p]])
        m["msk"] = msk_by_p[p]
        in_maps.append(m)

    nc = _build()
    trace = bool(os.environ.get("KERNEL_TRACE"))
    res = bass_utils.run_bass_kernel_spmd(
        nc, in_maps, list(range(NCORES)), trace=trace
    )
    LAST_RESULTS = res

    x_out = np.empty((B, S, E), np.float32)
    att_w = np.empty((B, S, S), np.float32)
    for c in range(NCORES):
        b, p = c // 2, c % 2
        rows = rows_by_p[p]
        x_out[b][rows] = res.results[c]["y_out"]
        att_w[b][rows] = res.results[c]["aw_out"]
    return (x_out, att_w)


# revision 12
# speedup vs baseline: 1.4829x; 1.0031x over previous
"""Trainium2 Bass kernel for a pre-LN transformer block (B=4,S=1024,E=1024,H=16).

Sharding: 8 cores = (batch b, parity p). Core (b,p) computes the full block for
batch b restricted to query tokens {q : q % 2 == p} (512 tokens). K/V are
computed for all 1024 tokens of the batch (duplicated across the pair), which
makes every core's instruction stream identical (pure SPMD) with per-core
differences carried entirely by input data (x slices + boundary masks).

Local token order: tile t in 0..3 holds global queries 256*(3-t)+2j+p, so the
causal context per tile is 1024-256t and AV matmul N shrinks monotonically.
"""

import math
import os

import numpy as np

B, S, E, H = 4, 1024, 1024, 16
DH = E // H
FF = 4 * E
P = 128
NCORES = 8
LN_EPS = 1e-5
NEG = -30000.0

_BUILD_CACHE = {}
LAST_RESULTS = None


def _build():
    if "nc" in _BUILD_CACHE:
        return _BUILD_CACHE["nc"]

    import concourse.bacc as bacc
    import concourse.bass as bass
    import concourse.mybir as mybir
    import concourse.tile as tile

    f32 = mybir.dt.float32
    bf16 = mybir.dt.bfloat16
    AF = mybir.ActivationFunctionType
    ALU = mybir.AluOpType

    nc = bacc.Bacc(
        "TRN2",
        target_bir_lowering=False,
        debug=False,
        enable_asserts=False,
        num_devices=NCORES,
    )

    # ---- DRAM I/O ----
    xg_d = nc.dram_tensor("x_glob", [S, E], f32, kind="ExternalInput")
    xl_d = nc.dram_tensor("x_loc", [S // 2, E], f32, kind="ExternalInput")
    wq_d = nc.dram_tensor("wq_sb", [P, 8, E], bf16, kind="ExternalInput")
    wk_d = nc.dram_tensor("wk_sb", [P, 8, E], bf16, kind="ExternalInput")
    wv_d = nc.dram_tensor("wv_sb", [P, 8, E], bf16, kind="ExternalInput")
    wo_d = nc.dram_tensor("wo_sb", [P, 8, E], bf16, kind="ExternalInput")
    wf_d = nc.dram_tensor("wf_sb", [8, P, 4, 8, P], bf16, kind="ExternalInput")
    wpa_d = nc.dram_tensor("wpa_sb", [8, P, 4, 512], bf16, kind="ExternalInput")
    wpb_d = nc.dram_tensor("wpb_sb", [8, P, 4, 512], bf16, kind="ExternalInput")
    bq_d = nc.dram_tensor("bq_sb", [P, 8], f32, kind="ExternalInput")
    bk_d = nc.dram_tensor("bk_sb", [P, 8], f32, kind="ExternalInput")
    bfc_d = nc.dram_tensor("bfc_sb", [P, 32], f32, kind="ExternalInput")
    bvec_d = nc.dram_tensor("bvec", [E], f32, kind="ExternalInput")
    bvec2_d = nc.dram_tensor("bvec2", [E], f32, kind="ExternalInput")
    msk_d = nc.dram_tensor("msk", [P, 4, 256], f32, kind="ExternalInput")
    yout_d = nc.dram_tensor("y_out", [S // 2, E], f32, kind="ExternalOutput")
    awout_d = nc.dram_tensor("aw_out", [S // 2, E], f32, kind="ExternalOutput")

    xg = xg_d.ap()
    xl = xl_d.ap()
    yout = yout_d.ap()
    awout = awout_d.ap()

    def bcast_ap(handle):
        ap = handle.ap()
        return bass.AP(tensor=ap.tensor, offset=ap.offset, ap=[[0, P]] + list(ap.ap))

    with tile.TileContext(nc) as tc:
        import contextlib

        stack = contextlib.ExitStack()
        with stack:
            persist = stack.enter_context(tc.tile_pool(name="persist", bufs=1))

            xres = persist.tile([P, 4, E], f32, tag="xres")
            ysb = persist.tile([P, 4, E], f32, tag="ysb")
            awsb = persist.tile([P, 4, S], f32, tag="awsb")
            awv = persist.tile([P, 4, S], bf16, tag="awv")
            z2T = persist.tile([P, 8, 512], bf16, tag="z2T")
            bvecb = persist.tile([P, E], f32, tag="bvecb")
            bvec2b = persist.tile([P, E], f32, tag="bvec2b")
            mskb = persist.tile([P, 4, 256], f32, tag="mskb")
            bqsb = persist.tile([P, 8], f32, tag="bqsb")
            bksb = persist.tile([P, 8], f32, tag="bksb")
            bfcsb = persist.tile([P, 32], f32, tag="bfcsb")
            epsb = persist.tile([P, 1], f32, tag="epsb")

            # constants
            nc.vector.memset(epsb[:], LN_EPS)
            nc.sync.dma_start(out=bqsb[:], in_=bq_d.ap())
            nc.sync.dma_start(out=bksb[:], in_=bk_d.ap())
            nc.sync.dma_start(out=bfcsb[:], in_=bfc_d.ap())
            nc.gpsimd.dma_start(out=mskb[:], in_=msk_d.ap())
            nc.gpsimd.dma_start(out=bvecb[:], in_=bcast_ap(bvec_d))
            nc.gpsimd.dma_start(out=bvec2b[:], in_=bcast_ap(bvec2_d))
            nc.vector.memset(awsb[:], 0.0)
            nc.vector.memset(awv[:], 0.0)

            # attention-scope persistent tensors (freed before MLP)
            import contextlib as _ctl
            astack = _ctl.ExitStack()
            apool = astack.enter_context(tc.tile_pool(name="attnbuf", bufs=1))
            hstack = _ctl.ExitStack()
            hpool = hstack.enter_context(tc.tile_pool(name="hbuf", bufs=1))
            hT = hpool.tile([P, 8, S], bf16, tag="hT")
            hqT = apool.tile([P, 8, 512], bf16, tag="hqT")
            kdt = apool.tile([P, 8, S], bf16, tag="kdt")
            vtd = apool.tile([P, 8, E], bf16, tag="vtd")
            qdt = apool.tile([P, 8, 512], bf16, tag="qdt")
            attsb = apool.tile([P, 8, 512], bf16, tag="attsb")

            # ---------- Phase A: LayerNorm1 (batched sweeps) + transpose ----------
            with tc.tile_pool(name="lnp", bufs=3) as lnp, tc.tile_pool(
                name="lnx", bufs=4
            ) as lnxp, tc.tile_pool(name="lns", bufs=2) as lns:

                _lnctr = [0]

                def ln_batch(srcs, dsts):
                    nb = len(srcs)
                    _lnctr[0] += 1
                    xts = []
                    mvb = lns.tile([P, nb, 2], f32, tag="ln_mv", name=f"mv{_lnctr[0]}")
                    for t in range(nb):
                        xt = lnxp.tile([P, E], f32, tag="ln_x")
                        nc.gpsimd.dma_start(out=xt[:], in_=srcs[t])
                        xts.append(xt)
                        st = lns.tile([P, 2, 6], f32, tag="ln_st", bufs=4)
                        nc.vector.bn_stats(out=st[:, 0, :], in_=xt[:, 0:512])
                        nc.vector.bn_stats(out=st[:, 1, :], in_=xt[:, 512:1024])
                        nc.vector.bn_aggr(out=mvb[:, t, :], in_=st[:])
                    sdb = lns.tile([P, nb], f32, tag="ln_sd")
                    nc.scalar.activation(
                        sdb[:], mvb[:, :, 1], AF.Sqrt, bias=epsb[:], scale=1.0
                    )
                    rsb = lns.tile([P, nb], f32, tag="ln_rs")
                    nc.vector.reciprocal(rsb[:], sdb[:])
                    nmub = lns.tile([P, nb], f32, tag="ln_nmu")
                    nc.vector.tensor_tensor(nmub[:], mvb[:, :, 0], rsb[:], op=ALU.mult)
                    nc.vector.tensor_scalar_mul(nmub[:], nmub[:], -1.0)
                    for t in range(nb):
                        zt = lnp.tile([P, E], bf16, tag="ln_z")
                        nc.scalar.activation(
                            zt[:],
                            xts[t][:],
                            AF.Identity,
                            bias=nmub[:, t : t + 1],
                            scale=rsb[:, t : t + 1],
                        )
                        nc.sync.dma_start_transpose(out=dsts[t], in_=zt[:])

                ln_batch(
                    [xl[P * t : P * (t + 1), :] for t in range(4)],
                    [hqT[:, :, P * t : P * (t + 1)] for t in range(4)],
                )
                ln_batch(
                    [xg[P * t : P * (t + 1), :] for t in range(4)],
                    [hT[:, :, P * t : P * (t + 1)] for t in range(4)],
                )
                ln_batch(
                    [xg[P * t : P * (t + 1), :] for t in range(4, 8)],
                    [hT[:, :, P * t : P * (t + 1)] for t in range(4, 8)],
                )
                for m in range(4):
                    nc.gpsimd.dma_start(
                        out=xres[:, m, :], in_=xl[P * m : P * (m + 1), :]
                    )
                    nc.vector.tensor_add(xres[:, m, :], xres[:, m, :], bvecb[:])

            # ---------- Phase B: QKV projections ----------
            with tc.tile_pool(name="wqkv", bufs=2) as wp, tc.tile_pool(
                name="psqkv", bufs=4, space="PSUM"
            ) as pp:
                wq_sb = wp.tile([P, 8, E], bf16, tag="w")
                nc.gpsimd.dma_start(out=wq_sb[:], in_=wq_d.ap())
                for oc in range(8):
                    ps = pp.tile([P, 512], f32, tag="ps")
                    for kc in range(8):
                        nc.tensor.matmul(
                            ps[:],
                            lhsT=wq_sb[:, kc, P * oc : P * (oc + 1)],
                            rhs=hqT[:, kc, :],
                            start=(kc == 0),
                            stop=(kc == 7),
                        )
                    nc.scalar.activation(
                        qdt[:, oc, :],
                        ps[:],
                        AF.Identity,
                        bias=bqsb[:, oc : oc + 1],
                        scale=1.0,
                    )
                wk_sb = wp.tile([P, 8, E], bf16, tag="w")
                nc.gpsimd.dma_start(out=wk_sb[:], in_=wk_d.ap())
                for oc in range(8):
                    for ch in range(2):
                        ps = pp.tile([P, 512], f32, tag="ps")
                        for kc in range(8):
                            nc.tensor.matmul(
                                ps[:],
                                lhsT=wk_sb[:, kc, P * oc : P * (oc + 1)],
                                rhs=hT[:, kc, 512 * ch : 512 * (ch + 1)],
                                start=(kc == 0),
                                stop=(kc == 7),
                            )
                        nc.scalar.activation(
                            kdt[:, oc, 512 * ch : 512 * (ch + 1)],
                            ps[:],
                            AF.Identity,
                            bias=bksb[:, oc : oc + 1],
                            scale=1.0,
                        )
                wv_sb = wp.tile([P, 8, E], bf16, tag="w")
                nc.gpsimd.dma_start(out=wv_sb[:], in_=wv_d.ap())
                for tkt in range(8):
                    for ch in range(2):
                        ps = pp.tile([P, 512], f32, tag="ps")
                        for kc in range(8):
                            nc.tensor.matmul(
                                ps[:],
                                lhsT=hT[:, kc, P * tkt : P * (tkt + 1)],
                                rhs=wv_sb[:, kc, 512 * ch : 512 * (ch + 1)],
                                start=(kc == 0),
                                stop=(kc == 7),
                            )
                        nc.vector.tensor_copy(
                            vtd[:, tkt, 512 * ch : 512 * (ch + 1)], ps[:]
                        )

            hstack.close()

            # ---------- Phase C: attention ----------
            NK = [512, 512, 384, 384, 256, 256, 128, 128]
            with tc.tile_pool(name="wo", bufs=1) as wop:
                wo_sb = wop.tile([P, 8, E], bf16, tag="wo")
                nc.gpsimd.dma_start(out=wo_sb[:], in_=wo_d.ap())
                with tc.tile_pool(name="prb", bufs=2) as prp, tc.tile_pool(
                    name="psc", bufs=3, space="PSUM"
                ) as scp, tc.tile_pool(
                    name="psav", bufs=2, space="PSUM"
                ) as avp, tc.tile_pool(name="asm", bufs=8) as smp:

                    def emit_av(i, pTs):
                        avps = avp.tile([P, 512], f32, tag="av", name=f"av{i}")
                        for kc in range(8):
                            nk = NK[kc]
                            for sub in range(2):
                                h = 2 * i + sub
                                nc.tensor.matmul(
                                    avps[64 * sub : 64 * (sub + 1), :nk],
                                    lhsT=vtd[:, kc, 64 * h : 64 * (h + 1)],
                                    rhs=pTs[sub][:, kc, :nk],
                                    start=(kc == 0),
                                    stop=(kc == 7),
                                    skip_group_check=True,
                                )
                        nc.vector.tensor_copy(attsb[:, i, :], avps[:])

                    pend = None
                    for i in range(8):
                        pTs = [
                            prp.tile(
                                [P, 8, 512], bf16, tag="pT", name=f"pT{i}_{s}", bufs=4
                            )
                            for s in range(2)
                        ]
                        for tl in range(4):
                            ctx = S - 256 * tl
                            sc2 = [
                                scp.tile([P, S], f32, tag="sc", name=f"sc{i}_{tl}_{s}")
                                for s in range(2)
                            ]
                            for c0 in range(0, ctx, 512):
                                n = min(512, ctx - c0)
                                for sub in range(2):
                                    nc.tensor.matmul(
                                        sc2[sub][:, c0 : c0 + n],
                                        lhsT=qdt[
                                            64 * sub : 64 * (sub + 1),
                                            i,
                                            P * tl : P * (tl + 1),
                                        ],
                                        rhs=kdt[
                                            64 * sub : 64 * (sub + 1), i, c0 : c0 + n
                                        ],
                                        start=True,
                                        stop=True,
                                    )
                            for sub in range(2):
                                sc = sc2[sub]
                                nc.vector.tensor_add(
                                    sc[:, ctx - 256 : ctx],
                                    sc[:, ctx - 256 : ctx],
                                    mskb[:, tl, :],
                                )
                                pf = prp.tile([P, S], f32, tag="pf")
                                den = smp.tile([P, 1], f32, tag="den")
                                nc.scalar.activation(
                                    pf[:, :ctx],
                                    sc[:, :ctx],
                                    AF.Exp,
                                    accum_out=den[:],
                                )
                                r = smp.tile([P, 1], f32, tag="r")
                                nc.vector.reciprocal(r[:], den[:])
                                pbt = prp.tile([P, S], bf16, tag="pb", bufs=3)
                                if sub == 0:
                                    nc.vector.tensor_scalar(
                                        out=pbt[:, :ctx],
                                        in0=pf[:, :ctx],
                                        scalar1=r[:],
                                        scalar2=None,
                                        op0=ALU.mult,
                                    )
                                    nc.gpsimd.tensor_tensor(
                                        awsb[:, tl, :ctx],
                                        awsb[:, tl, :ctx],
                                        pbt[:, :ctx],
                                        op=ALU.add,
                                    )
                                else:
                                    nc.scalar.activation(
                                        pbt[:, :ctx],
                                        pf[:, :ctx],
                                        AF.Copy,
                                        scale=r[:],
                                    )
                                    nc.vector.tensor_tensor(
                                        awv[:, tl, :ctx],
                                        awv[:, tl, :ctx],
                                        pbt[:, :ctx],
                                        op=ALU.add,
                                    )
                                nc.sync.dma_start_transpose(
                                    out=pTs[sub][:, : ctx // P, P * tl : P * (tl + 1)],
                                    in_=pbt[:, :ctx],
                                )
                        if pend is not None:
                            emit_av(*pend)
                        pend = (i, pTs)
                    emit_av(*pend)

                # ---------- Phase D: out_proj + residual ----------
                with tc.tile_pool(name="pso", bufs=2, space="PSUM") as pop:
                    for m in range(4):
                        for o2 in range(2):
                            ps = pop.tile([P, 512], f32, tag="po")
                            for i in range(8):
                                nc.tensor.matmul(
                                    ps[:],
                                    lhsT=attsb[:, i, P * m : P * (m + 1)],
                                    rhs=wo_sb[:, i, 512 * o2 : 512 * (o2 + 1)],
                                    start=(i == 0),
                                    stop=(i == 7),
                                )
                            nc.vector.tensor_add(
                                ysb[:, m, 512 * o2 : 512 * (o2 + 1)],
                                ps[:],
                                xres[:, m, 512 * o2 : 512 * (o2 + 1)],
                            )

            # ---------- Phase E: LayerNorm2 + transpose ----------
            with tc.tile_pool(name="ln2p", bufs=3) as lnp2, tc.tile_pool(
                name="ln2s", bufs=4
            ) as lns2:
                for m in range(4):
                    st = lns2.tile([P, 2, 6], f32, tag="l2st")
                    nc.vector.bn_stats(out=st[:, 0, :], in_=ysb[:, m, 0:512])
                    nc.vector.bn_stats(out=st[:, 1, :], in_=ysb[:, m, 512:1024])
                    mv = lns2.tile([P, 2], f32, tag="l2mv")
                    nc.vector.bn_aggr(out=mv[:], in_=st[:])
                    sd = lns2.tile([P, 1], f32, tag="l2sd")
                    nc.scalar.activation(
                        sd[:], mv[:, 1:2], AF.Sqrt, bias=epsb[:], scale=1.0
                    )
                    rs = lns2.tile([P, 1], f32, tag="l2rs")
                    nc.vector.reciprocal(rs[:], sd[:])
                    nmu = lns2.tile([P, 1], f32, tag="l2nmu")
                    nc.vector.tensor_scalar(
                        out=nmu[:], in0=mv[:, 0:1], scalar1=rs[:], scalar2=-1.0,
                        op0=ALU.mult, op1=ALU.mult,
                    )
                    z2 = lnp2.tile([P, E], bf16, tag="l2z")
                    nc.scalar.activation(
                        z2[:], ysb[:, m, :], AF.Identity, bias=nmu[:], scale=rs[:]
                    )
                    nc.sync.dma_start_transpose(
                        out=z2T[:, :, P * m : P * (m + 1)], in_=z2[:]
                    )

            # close attention buffers before MLP
            astack.close()

            # ---------- Phase F/G: MLP ----------
            with tc.tile_pool(name="mlp", bufs=1) as mp, tc.tile_pool(
                name="blk", bufs=2
            ) as bp, tc.tile_pool(name="x3p", bufs=3) as xp, tc.tile_pool(
                name="pspj", bufs=1, space="PSUM"
            ) as jp:
                hidT = mp.tile([P, 32, 512], bf16, tag="hidT")
                psA = [
                    jp.tile([P, 512], f32, tag=f"pja{m}", name=f"pja{m}")
                    for m in range(4)
                ]
                wfap = wf_d.ap()
                wpaap = wpa_d.ap()
                wpbap = wpb_d.ap()
                with tc.tile_pool(name="psf", bufs=2, space="PSUM") as fp:
                    for hg in range(8):
                        bf4 = bp.tile([P, 4, 8, P], bf16, tag="bf4")
                        nc.gpsimd.dma_start(out=bf4[:], in_=wfap[hg])
                        bpa4 = bp.tile([P, 4, 512], bf16, tag="bpa4")
                        nc.gpsimd.dma_start(out=bpa4[:], in_=wpaap[hg])
                        for hi in range(4):
                            hc = 4 * hg + hi
                            psF = fp.tile([P, 512], f32, tag="psF")
                            for kc in range(8):
                                nc.tensor.matmul(
                                    psF[:],
                                    lhsT=bf4[:, hi, kc, :],
                                    rhs=z2T[:, kc, :],
                                    start=(kc == 0),
                                    stop=(kc == 7),
                                )
                            nc.scalar.activation(
                                hidT[:, hc, :],
                                psF[:],
                                AF.Gelu_apprx_tanh,
                                bias=bfcsb[:, hc : hc + 1],
                                scale=1.0,
                            )
                            for m in range(4):
                                nc.tensor.matmul(
                                    psA[m][:],
                                    lhsT=hidT[:, hc, P * m : P * (m + 1)],
                                    rhs=bpa4[:, hi, :],
                                    start=(hc == 0),
                                    stop=(hc == 31),
                                )
                    for m in range(4):
                        x3 = xp.tile([P, 512], f32, tag="x3")
                        nc.vector.tensor_add(x3[:], psA[m][:], ysb[:, m, 0:512])
                        nc.vector.tensor_add(x3[:], x3[:], bvec2b[:, 0:512])
                        nc.gpsimd.dma_start(
                            out=yout[P * m : P * (m + 1), 0:512], in_=x3[:]
                        )
                # att_weights finalize: merge halves, scale by 1/H, store
                for tl in range(4):
                    nc.vector.tensor_tensor(
                        awsb[:, tl, :], awsb[:, tl, :], awv[:, tl, :], op=ALU.add
                    )
                    nc.vector.tensor_scalar_mul(
                        awsb[:, tl, :], awsb[:, tl, :], 1.0 / H
                    )
                    nc.gpsimd.dma_start(
                        out=awout[P * tl : P * (tl + 1), :], in_=awsb[:, tl, :]
                    )
                with tc.tile_pool(name="pspjb", bufs=1, space="PSUM") as jpb:
                    psB = [
                        jpb.tile([P, 512], f32, tag=f"pjb{m}", name=f"pjb{m}")
                        for m in range(4)
                    ]
                    for hg in range(8):
                        bpb4 = bp.tile([P, 4, 512], bf16, tag="bpb4")
                        nc.gpsimd.dma_start(out=bpb4[:], in_=wpbap[hg])
                        for hi in range(4):
                            hc = 4 * hg + hi
                            for m in range(4):
                                nc.tensor.matmul(
                                    psB[m][:],
                                    lhsT=hidT[:, hc, P * m : P * (m + 1)],
                                    rhs=bpb4[:, hi, :],
                                    start=(hc == 0),
                                    stop=(hc == 31),
                                )
                    for m in range(4):
                        x3 = xp.tile([P, 512], f32, tag="x3")
                        nc.vector.tensor_add(x3[:], psB[m][:], ysb[:, m, 512:1024])
                        nc.vector.tensor_add(x3[:], x3[:], bvec2b[:, 512:1024])
                        nc.gpsimd.dma_start(
                            out=yout[P * m : P * (m + 1), 512:1024], in_=x3[:]
                        )

    nc.compile()
    _BUILD_CACHE["nc"] = nc
    return nc


def _local_rows(p):
    rows = []
    for t in range(4):
        g = 3 - t
        rows.extend(256 * g + 2 * j + p for j in range(P))
    return np.array(rows, dtype=np.int64)


def kernel(
    x,
    causal_mask,
    ln1_g,
    ln1_b,
    ln2_g,
    ln2_b,
    w_in,
    b_in,
    w_out,
    b_out,
    w_fc,
    b_fc,
    w_proj,
    b_proj,
):
    global LAST_RESULTS
    import ml_dtypes

    from concourse import bass_utils

    bf = ml_dtypes.bfloat16
    x = np.asarray(x, np.float32)
    causal_mask = np.asarray(causal_mask, np.float32)
    f32 = lambda a: np.ascontiguousarray(np.asarray(a, np.float32))
    ln1_g, ln1_b, ln2_g, ln2_b = map(f32, (ln1_g, ln1_b, ln2_g, ln2_b))
    w_in, b_in, w_out, b_out = map(f32, (w_in, b_in, w_out, b_out))
    w_fc, b_fc, w_proj, b_proj = map(f32, (w_fc, b_fc, w_proj, b_proj))

    wq, wk, wv = w_in[:E], w_in[E : 2 * E], w_in[2 * E :]
    bq, bk, bv = b_in[:E], b_in[E : 2 * E], b_in[2 * E :]
    scale = 1.0 / math.sqrt(DH)

    wq2 = (wq * ln1_g[None, :]) * scale
    bq2 = (wq @ ln1_b + bq) * scale
    wk2 = wk * ln1_g[None, :]
    bk2 = wk @ ln1_b + bk
    wv2 = wv * ln1_g[None, :]
    bv2 = wv @ ln1_b + bv
    bvec = b_out + w_out @ bv2
    wf2 = w_fc * ln2_g[None, :]
    bfc2 = b_fc + w_fc @ ln2_b
    bvec2 = b_proj

    def tile_w(wT2):
        # [out,in] weight -> SBUF layout [p, kc, o] with p the contraction row
        return np.ascontiguousarray(
            wT2.T.reshape(8, P, E).transpose(1, 0, 2).astype(bf)
        )

    wq_sb = tile_w(wq2)
    wk_sb = tile_w(wk2)
    wv_sb = tile_w(wv2)
    wo_sb = tile_w(w_out)
    # w_fc': [FF, E] -> [hg, p(k), hi, kc, c] so each hg slice is one DMA
    wf_sb = np.ascontiguousarray(
        wf2.reshape(8, 4, P, 8, P).transpose(0, 4, 1, 3, 2).astype(bf)
    )
    wpT = np.ascontiguousarray(w_proj.T)  # [FF, E]
    # [hg, p(h-row), hi, c]
    wpa_sb = np.ascontiguousarray(
        wpT[:, :512].reshape(8, 4, P, 512).transpose(0, 2, 1, 3).astype(bf)
    )
    wpb_sb = np.ascontiguousarray(
        wpT[:, 512:].reshape(8, 4, P, 512).transpose(0, 2, 1, 3).astype(bf)
    )
    bq_sb = np.ascontiguousarray(bq2.reshape(8, P).T)
    bk_sb = np.ascontiguousarray(bk2.reshape(8, P).T)
    bfc_sb = np.ascontiguousarray(bfc2.reshape(32, P).T)

    cm = np.maximum(causal_mask, NEG)

    in_maps = []
    rows_by_p = [_local_rows(0), _local_rows(1)]
    msk_by_p = []
    for p in range(2):
        msk = np.empty((P, 4, 256), np.float32)
        for t in range(4):
            ctx = S - 256 * t
            rows_t = rows_by_p[p][P * t : P * (t + 1)]
            msk[:, t, :] = cm[rows_t, ctx - 256 : ctx]
        msk_by_p.append(msk)

    shared = dict(
        wq_sb=wq_sb,
        wk_sb=wk_sb,
        wv_sb=wv_sb,
        wo_sb=wo_sb,
        wf_sb=wf_sb,
        wpa_sb=wpa_sb,
        wpb_sb=wpb_sb,
        bq_sb=bq_sb,
        bk_sb=bk_sb,
        bfc_sb=bfc_sb,
        bvec=np.ascontiguousarray(bvec),
        bvec2=np.ascontiguousarray(bvec2),
    )
    for c in range(NCORES):
        b, p = c // 2, c % 2
        m = dict(shared)
        m["x_glob"] = np.ascontiguousarray(x[b])
        m["x_loc"] = np.ascontiguousarray(x[b][rows_by_p[p]])
        m["msk"] = msk_by_p[p]
        in_maps.append(m)

    nc = _build()
    trace = bool(os.environ.get("KERNEL_TRACE"))
    res = bass_utils.run_bass_kernel_spmd(
        nc, in_maps, list(range(NCORES)), trace=trace
    )
    LAST_RESULTS = res

    x_out = np.empty((B, S, E), np.float32)
    att_w = np.empty((B, S, S), np.float32)
    for c in range(NCORES):
        b, p = c // 2, c % 2
        rows = rows_by_p[p]
        x_out[b][rows] = res.results[c]["y_out"]
        att_w[b][rows] = res.results[c]["aw_out"]
    return (x_out, att_w)


# revision 13
# speedup vs baseline: 1.5851x; 1.0689x over previous
"""Trainium2 Bass kernel for a pre-LN transformer block (B=4,S=1024,E=1024,H=16).

Sharding: 8 cores = (batch b, parity p). Core (b,p) computes the full block for
batch b restricted to query tokens {q : q % 2 == p} (512 tokens). K/V are
computed for all 1024 tokens of the batch (duplicated across the pair), which
makes every core's instruction stream identical (pure SPMD) with per-core
differences carried entirely by input data (x slices + boundary masks).

Local token order: tile t in 0..3 holds global queries 256*(3-t)+2j+p, so the
causal context per tile is 1024-256t and AV matmul N shrinks monotonically.
"""

import math
import os

import numpy as np

B, S, E, H = 4, 1024, 1024, 16
DH = E // H
FF = 4 * E
P = 128
NCORES = 8
LN_EPS = 1e-5
NEG = -30000.0

_BUILD_CACHE = {}
LAST_RESULTS = None


def _build():
    if "nc" in _BUILD_CACHE:
        return _BUILD_CACHE["nc"]

    import concourse.bacc as bacc
    import concourse.bass as bass
    import concourse.mybir as mybir
    import concourse.tile as tile

    f32 = mybir.dt.float32
    bf16 = mybir.dt.bfloat16
    AF = mybir.ActivationFunctionType
    ALU = mybir.AluOpType

    nc = bacc.Bacc(
        "TRN2",
        target_bir_lowering=False,
        debug=False,
        enable_asserts=False,
        num_devices=NCORES,
    )

    # ---- DRAM I/O ----
    xg_d = nc.dram_tensor("x_glob_bf", [S, E], bf16, kind="ExternalInput")
    xl_d = nc.dram_tensor("x_loc", [S // 2, E], f32, kind="ExternalInput")
    xlb_d = nc.dram_tensor("x_loc_bf", [S // 2, E], bf16, kind="ExternalInput")
    wq_d = nc.dram_tensor("wq_sb", [P, 8, E], bf16, kind="ExternalInput")
    wk_d = nc.dram_tensor("wk_sb", [P, 8, E], bf16, kind="ExternalInput")
    wv_d = nc.dram_tensor("wv_sb", [P, 8, E], bf16, kind="ExternalInput")
    wo_d = nc.dram_tensor("wo_sb", [P, 8, E], bf16, kind="ExternalInput")
    wf_d = nc.dram_tensor("wf_sb", [8, P, 4, 8, P], bf16, kind="ExternalInput")
    wpa_d = nc.dram_tensor("wpa_sb", [8, P, 4, 512], bf16, kind="ExternalInput")
    wpb_d = nc.dram_tensor("wpb_sb", [8, P, 4, 512], bf16, kind="ExternalInput")
    bq_d = nc.dram_tensor("bq_sb", [P, 8], f32, kind="ExternalInput")
    bk_d = nc.dram_tensor("bk_sb", [P, 8], f32, kind="ExternalInput")
    bfc_d = nc.dram_tensor("bfc_sb", [P, 32], f32, kind="ExternalInput")
    bvec_d = nc.dram_tensor("bvec", [E], f32, kind="ExternalInput")
    bvec2_d = nc.dram_tensor("bvec2", [E], f32, kind="ExternalInput")
    msk_d = nc.dram_tensor("msk", [P, 4, 256], f32, kind="ExternalInput")
    yout_d = nc.dram_tensor("y_out", [S // 2, E], f32, kind="ExternalOutput")
    awout_d = nc.dram_tensor("aw_out", [S // 2, E], f32, kind="ExternalOutput")

    xg = xg_d.ap()
    xl = xl_d.ap()
    xlb = xlb_d.ap()
    yout = yout_d.ap()
    awout = awout_d.ap()

    def bcast_ap(handle):
        ap = handle.ap()
        return bass.AP(tensor=ap.tensor, offset=ap.offset, ap=[[0, P]] + list(ap.ap))

    with tile.TileContext(nc) as tc:
        import contextlib

        stack = contextlib.ExitStack()
        with stack:
            persist = stack.enter_context(tc.tile_pool(name="persist", bufs=1))

            xres = persist.tile([P, 4, E], f32, tag="xres")
            ysb = persist.tile([P, 4, E], f32, tag="ysb")
            awsb = persist.tile([P, 4, S], f32, tag="awsb")
            awv = persist.tile([P, 4, S], bf16, tag="awv")
            z2T = persist.tile([P, 8, 512], bf16, tag="z2T")
            bvecb = persist.tile([P, E], f32, tag="bvecb")
            bvec2b = persist.tile([P, E], f32, tag="bvec2b")
            mskb = persist.tile([P, 4, 256], f32, tag="mskb")
            bqsb = persist.tile([P, 8], f32, tag="bqsb")
            bksb = persist.tile([P, 8], f32, tag="bksb")
            bfcsb = persist.tile([P, 32], f32, tag="bfcsb")
            epsb = persist.tile([P, 1], f32, tag="epsb")

            # constants
            nc.vector.memset(epsb[:], LN_EPS)
            nc.sync.dma_start(out=bqsb[:], in_=bq_d.ap())
            nc.sync.dma_start(out=bksb[:], in_=bk_d.ap())
            nc.sync.dma_start(out=bfcsb[:], in_=bfc_d.ap())
            nc.gpsimd.dma_start(out=mskb[:], in_=msk_d.ap())
            nc.gpsimd.dma_start(out=bvecb[:], in_=bcast_ap(bvec_d))
            nc.gpsimd.dma_start(out=bvec2b[:], in_=bcast_ap(bvec2_d))
            nc.vector.memset(awsb[:], 0.0)
            nc.vector.memset(awv[:], 0.0)

            # attention-scope persistent tensors (freed before MLP)
            import contextlib as _ctl
            astack = _ctl.ExitStack()
            apool = astack.enter_context(tc.tile_pool(name="attnbuf", bufs=1))
            hstack = _ctl.ExitStack()
            hpool = hstack.enter_context(tc.tile_pool(name="hbuf", bufs=1))
            hT = hpool.tile([P, 8, S], bf16, tag="hT")
            hqT = apool.tile([P, 8, 512], bf16, tag="hqT")
            kdt = apool.tile([P, 8, S], bf16, tag="kdt")
            vtd = apool.tile([P, 8, E], bf16, tag="vtd")
            qdt = apool.tile([P, 8, 512], bf16, tag="qdt")
            attsb = apool.tile([P, 8, 512], bf16, tag="attsb")

            # ---------- Phase A: LayerNorm1 (batched sweeps) + transpose ----------
            with tc.tile_pool(name="lnp", bufs=3) as lnp, tc.tile_pool(
                name="lnx", bufs=8
            ) as lnxp, tc.tile_pool(name="lns", bufs=2) as lns:

                _lnctr = [0]
                _dmae = [nc.gpsimd, nc.sync, nc.scalar]

                def ln_batch(srcs, dsts):
                    nb = len(srcs)
                    _lnctr[0] += 1
                    xts = []
                    mvb = lns.tile([P, nb, 2], f32, tag="ln_mv", name=f"mv{_lnctr[0]}")
                    for t in range(nb):
                        xt = lnxp.tile([P, E], bf16, tag="ln_x")
                        _dmae[t % 3].dma_start(out=xt[:], in_=srcs[t])
                        xts.append(xt)
                        st = lns.tile([P, 2, 6], f32, tag="ln_st", bufs=4)
                        nc.vector.bn_stats(out=st[:, 0, :], in_=xt[:, 0:512])
                        nc.vector.bn_stats(out=st[:, 1, :], in_=xt[:, 512:1024])
                        nc.vector.bn_aggr(out=mvb[:, t, :], in_=st[:])
                    sdb = lns.tile([P, nb], f32, tag="ln_sd")
                    nc.scalar.activation(
                        sdb[:], mvb[:, :, 1], AF.Sqrt, bias=epsb[:], scale=1.0
                    )
                    rsb = lns.tile([P, nb], f32, tag="ln_rs")
                    nc.vector.reciprocal(rsb[:], sdb[:])
                    nmub = lns.tile([P, nb], f32, tag="ln_nmu")
                    nc.vector.tensor_tensor(nmub[:], mvb[:, :, 0], rsb[:], op=ALU.mult)
                    nc.vector.tensor_scalar_mul(nmub[:], nmub[:], -1.0)
                    for t in range(nb):
                        zt = lnp.tile([P, E], bf16, tag="ln_z")
                        nc.scalar.activation(
                            zt[:],
                            xts[t][:],
                            AF.Identity,
                            bias=nmub[:, t : t + 1],
                            scale=rsb[:, t : t + 1],
                        )
                        nc.sync.dma_start_transpose(out=dsts[t], in_=zt[:])

                ln_batch(
                    [xlb[P * t : P * (t + 1), :] for t in range(4)],
                    [hqT[:, :, P * t : P * (t + 1)] for t in range(4)],
                )
                ln_batch(
                    [xg[P * t : P * (t + 1), :] for t in range(4)],
                    [hT[:, :, P * t : P * (t + 1)] for t in range(4)],
                )
                ln_batch(
                    [xg[P * t : P * (t + 1), :] for t in range(4, 8)],
                    [hT[:, :, P * t : P * (t + 1)] for t in range(4, 8)],
                )
                for m in range(4):
                    nc.gpsimd.dma_start(
                        out=xres[:, m, :], in_=xl[P * m : P * (m + 1), :]
                    )
                    nc.vector.tensor_add(xres[:, m, :], xres[:, m, :], bvecb[:])

            # ---------- Phase B: QKV projections ----------
            with tc.tile_pool(name="wqkv", bufs=2) as wp, tc.tile_pool(
                name="psqkv", bufs=4, space="PSUM"
            ) as pp:
                wq_sb = wp.tile([P, 8, E], bf16, tag="w")
                nc.scalar.dma_start(out=wq_sb[:], in_=wq_d.ap())
                for oc in range(8):
                    ps = pp.tile([P, 512], f32, tag="ps")
                    for kc in range(8):
                        nc.tensor.matmul(
                            ps[:],
                            lhsT=wq_sb[:, kc, P * oc : P * (oc + 1)],
                            rhs=hqT[:, kc, :],
                            start=(kc == 0),
                            stop=(kc == 7),
                        )
                    nc.scalar.activation(
                        qdt[:, oc, :],
                        ps[:],
                        AF.Identity,
                        bias=bqsb[:, oc : oc + 1],
                        scale=1.0,
                    )
                wk_sb = wp.tile([P, 8, E], bf16, tag="w")
                nc.sync.dma_start(out=wk_sb[:], in_=wk_d.ap())
                for oc in range(8):
                    for ch in range(2):
                        ps = pp.tile([P, 512], f32, tag="ps")
                        for kc in range(8):
                            nc.tensor.matmul(
                                ps[:],
                                lhsT=wk_sb[:, kc, P * oc : P * (oc + 1)],
                                rhs=hT[:, kc, 512 * ch : 512 * (ch + 1)],
                                start=(kc == 0),
                                stop=(kc == 7),
                            )
                        nc.scalar.activation(
                            kdt[:, oc, 512 * ch : 512 * (ch + 1)],
                            ps[:],
                            AF.Identity,
                            bias=bksb[:, oc : oc + 1],
                            scale=1.0,
                        )
                wv_sb = wp.tile([P, 8, E], bf16, tag="w")
                nc.gpsimd.dma_start(out=wv_sb[:], in_=wv_d.ap())
                for tkt in range(8):
                    for ch in range(2):
                        ps = pp.tile([P, 512], f32, tag="ps")
                        for kc in range(8):
                            nc.tensor.matmul(
                                ps[:],
                                lhsT=hT[:, kc, P * tkt : P * (tkt + 1)],
                                rhs=wv_sb[:, kc, 512 * ch : 512 * (ch + 1)],
                                start=(kc == 0),
                                stop=(kc == 7),
                            )
                        nc.vector.tensor_copy(
                            vtd[:, tkt, 512 * ch : 512 * (ch + 1)], ps[:]
                        )

            hstack.close()

            # ---------- Phase C: attention ----------
            NK = [512, 512, 384, 384, 256, 256, 128, 128]
            with tc.tile_pool(name="wo", bufs=1) as wop:
                wo_sb = wop.tile([P, 8, E], bf16, tag="wo")
                nc.scalar.dma_start(out=wo_sb[:], in_=wo_d.ap())
                with tc.tile_pool(name="prb", bufs=2) as prp, tc.tile_pool(
                    name="psc", bufs=3, space="PSUM"
                ) as scp, tc.tile_pool(
                    name="psav", bufs=2, space="PSUM"
                ) as avp, tc.tile_pool(name="asm", bufs=8) as smp:

                    def emit_av(i, pTs):
                        avps = avp.tile([P, 512], f32, tag="av", name=f"av{i}")
                        for kc in range(8):
                            nk = NK[kc]
                            for sub in range(2):
                                h = 2 * i + sub
                                nc.tensor.matmul(
                                    avps[64 * sub : 64 * (sub + 1), :nk],
                                    lhsT=vtd[:, kc, 64 * h : 64 * (h + 1)],
                                    rhs=pTs[sub][:, kc, :nk],
                                    start=(kc == 0),
                                    stop=(kc == 7),
                                    skip_group_check=True,
                                )
                        nc.vector.tensor_copy(attsb[:, i, :], avps[:])

                    pend = None
                    for i in range(8):
                        pTs = [
                            prp.tile(
                                [P, 8, 512], bf16, tag="pT", name=f"pT{i}_{s}", bufs=4
                            )
                            for s in range(2)
                        ]
                        for tl in range(4):
                            ctx = S - 256 * tl
                            sc2 = [
                                scp.tile([P, S], f32, tag="sc", name=f"sc{i}_{tl}_{s}")
                                for s in range(2)
                            ]
                            for c0 in range(0, ctx, 512):
                                n = min(512, ctx - c0)
                                for sub in range(2):
                                    nc.tensor.matmul(
                                        sc2[sub][:, c0 : c0 + n],
                                        lhsT=qdt[
                                            64 * sub : 64 * (sub + 1),
                                            i,
                                            P * tl : P * (tl + 1),
                                        ],
                                        rhs=kdt[
                                            64 * sub : 64 * (sub + 1), i, c0 : c0 + n
                                        ],
                                        start=True,
                                        stop=True,
                                    )
                            for sub in range(2):
                                sc = sc2[sub]
                                nc.vector.tensor_add(
                                    sc[:, ctx - 256 : ctx],
                                    sc[:, ctx - 256 : ctx],
                                    mskb[:, tl, :],
                                )
                                pf = prp.tile([P, S], f32, tag="pf")
                                den = smp.tile([P, 1], f32, tag="den")
                                nc.scalar.activation(
                                    pf[:, :ctx],
                                    sc[:, :ctx],
                                    AF.Exp,
                                    accum_out=den[:],
                                )
                                r = smp.tile([P, 1], f32, tag="r")
                                nc.vector.reciprocal(r[:], den[:])
                                pbt = prp.tile([P, S], bf16, tag="pb", bufs=3)
                                if sub == 0:
                                    nc.vector.tensor_scalar(
                                        out=pbt[:, :ctx],
                                        in0=pf[:, :ctx],
                                        scalar1=r[:],
                                        scalar2=None,
                                        op0=ALU.mult,
                                    )
                                    nc.gpsimd.tensor_tensor(
                                        awsb[:, tl, :ctx],
                                        awsb[:, tl, :ctx],
                                        pbt[:, :ctx],
                                        op=ALU.add,
                                    )
                                else:
                                    nc.scalar.activation(
                                        pbt[:, :ctx],
                                        pf[:, :ctx],
                                        AF.Copy,
                                        scale=r[:],
                                    )
                                    nc.vector.tensor_tensor(
                                        awv[:, tl, :ctx],
                                        awv[:, tl, :ctx],
                                        pbt[:, :ctx],
                                        op=ALU.add,
                                    )
                                nc.sync.dma_start_transpose(
                                    out=pTs[sub][:, : ctx // P, P * tl : P * (tl + 1)],
                                    in_=pbt[:, :ctx],
                                )
                        if pend is not None:
                            emit_av(*pend)
                        pend = (i, pTs)
                    emit_av(*pend)

                # ---------- Phase D: out_proj + residual ----------
                with tc.tile_pool(name="pso", bufs=2, space="PSUM") as pop:
                    for m in range(4):
                        for o2 in range(2):
                            ps = pop.tile([P, 512], f32, tag="po")
                            for i in range(8):
                                nc.tensor.matmul(
                                    ps[:],
                                    lhsT=attsb[:, i, P * m : P * (m + 1)],
                                    rhs=wo_sb[:, i, 512 * o2 : 512 * (o2 + 1)],
                                    start=(i == 0),
                                    stop=(i == 7),
                                )
                            nc.vector.tensor_add(
                                ysb[:, m, 512 * o2 : 512 * (o2 + 1)],
                                ps[:],
                                xres[:, m, 512 * o2 : 512 * (o2 + 1)],
                            )

            # ---------- Phase E: LayerNorm2 + transpose ----------
            with tc.tile_pool(name="ln2p", bufs=3) as lnp2, tc.tile_pool(
                name="ln2s", bufs=4
            ) as lns2:
                for m in range(4):
                    st = lns2.tile([P, 2, 6], f32, tag="l2st")
                    nc.vector.bn_stats(out=st[:, 0, :], in_=ysb[:, m, 0:512])
                    nc.vector.bn_stats(out=st[:, 1, :], in_=ysb[:, m, 512:1024])
                    mv = lns2.tile([P, 2], f32, tag="l2mv")
                    nc.vector.bn_aggr(out=mv[:], in_=st[:])
                    sd = lns2.tile([P, 1], f32, tag="l2sd")
                    nc.scalar.activation(
                        sd[:], mv[:, 1:2], AF.Sqrt, bias=epsb[:], scale=1.0
                    )
                    rs = lns2.tile([P, 1], f32, tag="l2rs")
                    nc.vector.reciprocal(rs[:], sd[:])
                    nmu = lns2.tile([P, 1], f32, tag="l2nmu")
                    nc.vector.tensor_scalar(
                        out=nmu[:], in0=mv[:, 0:1], scalar1=rs[:], scalar2=-1.0,
                        op0=ALU.mult, op1=ALU.mult,
                    )
                    z2 = lnp2.tile([P, E], bf16, tag="l2z")
                    nc.scalar.activation(
                        z2[:], ysb[:, m, :], AF.Identity, bias=nmu[:], scale=rs[:]
                    )
                    nc.sync.dma_start_transpose(
                        out=z2T[:, :, P * m : P * (m + 1)], in_=z2[:]
                    )

            # close attention buffers before MLP
            astack.close()

            # ---------- Phase F/G: MLP ----------
            with tc.tile_pool(name="mlp", bufs=1) as mp, tc.tile_pool(
                name="blk", bufs=2
            ) as bp, tc.tile_pool(name="x3p", bufs=3) as xp, tc.tile_pool(
                name="pspj", bufs=1, space="PSUM"
            ) as jp:
                hidT = mp.tile([P, 32, 512], bf16, tag="hidT")
                psA = [
                    jp.tile([P, 512], f32, tag=f"pja{m}", name=f"pja{m}")
                    for m in range(4)
                ]
                wfap = wf_d.ap()
                wpaap = wpa_d.ap()
                wpbap = wpb_d.ap()
                with tc.tile_pool(name="psf", bufs=2, space="PSUM") as fp:
                    for hg in range(8):
                        bf4 = bp.tile([P, 4, 8, P], bf16, tag="bf4")
                        ([nc.scalar, nc.sync][hg % 2]).dma_start(out=bf4[:], in_=wfap[hg])
                        bpa4 = bp.tile([P, 4, 512], bf16, tag="bpa4")
                        nc.gpsimd.dma_start(out=bpa4[:], in_=wpaap[hg])
                        for hi in range(4):
                            hc = 4 * hg + hi
                            psF = fp.tile([P, 512], f32, tag="psF")
                            for kc in range(8):
                                nc.tensor.matmul(
                                    psF[:],
                                    lhsT=bf4[:, hi, kc, :],
                                    rhs=z2T[:, kc, :],
                                    start=(kc == 0),
                                    stop=(kc == 7),
                                )
                            nc.scalar.activation(
                                hidT[:, hc, :],
                                psF[:],
                                AF.Gelu_apprx_tanh,
                                bias=bfcsb[:, hc : hc + 1],
                                scale=1.0,
                            )
                            for m in range(4):
                                nc.tensor.matmul(
                                    psA[m][:],
                                    lhsT=hidT[:, hc, P * m : P * (m + 1)],
                                    rhs=bpa4[:, hi, :],
                                    start=(hc == 0),
                                    stop=(hc == 31),
                                )
                    for m in range(4):
                        x3 = xp.tile([P, 512], f32, tag="x3")
                        nc.vector.tensor_add(x3[:], psA[m][:], ysb[:, m, 0:512])
                        nc.vector.tensor_add(x3[:], x3[:], bvec2b[:, 0:512])
                        nc.gpsimd.dma_start(
                            out=yout[P * m : P * (m + 1), 0:512], in_=x3[:]
                        )
                # att_weights finalize: merge halves, scale by 1/H, store
                for tl in range(4):
                    nc.vector.tensor_tensor(
                        awsb[:, tl, :], awsb[:, tl, :], awv[:, tl, :], op=ALU.add
                    )
                    nc.vector.tensor_scalar_mul(
                        awsb[:, tl, :], awsb[:, tl, :], 1.0 / H
                    )
                    nc.gpsimd.dma_start(
                        out=awout[P * tl : P * (tl + 1), :], in_=awsb[:, tl, :]
                    )
                with tc.tile_pool(name="pspjb", bufs=1, space="PSUM") as jpb:
                    psB = [
                        jpb.tile([P, 512], f32, tag=f"pjb{m}", name=f"pjb{m}")
                        for m in range(4)
                    ]
                    for hg in range(8):
                        bpb4 = bp.tile([P, 4, 512], bf16, tag="bpb4")
                        ([nc.sync, nc.scalar][hg % 2]).dma_start(out=bpb4[:], in_=wpbap[hg])
                        for hi in range(4):
                            hc = 4 * hg + hi
                            for m in range(4):
                                nc.tensor.matmul(
                                    psB[m][:],
                                    lhsT=hidT[:, hc, P * m : P * (m + 1)],
                                    rhs=bpb4[:, hi, :],
                                    start=(hc == 0),
                                    stop=(hc == 31),
                                )
                    for m in range(4):
                        x3 = xp.tile([P, 512], f32, tag="x3")
                        nc.vector.tensor_add(x3[:], psB[m][:], ysb[:, m, 512:1024])
                        nc.vector.tensor_add(x3[:], x3[:], bvec2b[:, 512:1024])
                        nc.gpsimd.dma_start(
                            out=yout[P * m : P * (m + 1), 512:1024], in_=x3[:]
                        )

    nc.compile()
    _BUILD_CACHE["nc"] = nc
    return nc


def _local_rows(p):
    rows = []
    for t in range(4):
        g = 3 - t
        rows.extend(256 * g + 2 * j + p for j in range(P))
    return np.array(rows, dtype=np.int64)


def kernel(
    x,
    causal_mask,
    ln1_g,
    ln1_b,
    ln2_g,
    ln2_b,
    w_in,
    b_in,
    w_out,
    b_out,
    w_fc,
    b_fc,
    w_proj,
    b_proj,
):
    global LAST_RESULTS
    import ml_dtypes

    from concourse import bass_utils

    bf = ml_dtypes.bfloat16
    x = np.asarray(x, np.float32)
    causal_mask = np.asarray(causal_mask, np.float32)
    f32 = lambda a: np.ascontiguousarray(np.asarray(a, np.float32))
    ln1_g, ln1_b, ln2_g, ln2_b = map(f32, (ln1_g, ln1_b, ln2_g, ln2_b))
    w_in, b_in, w_out, b_out = map(f32, (w_in, b_in, w_out, b_out))
    w_fc, b_fc, w_proj, b_proj = map(f32, (w_fc, b_fc, w_proj, b_proj))

    wq, wk, wv = w_in[:E], w_in[E : 2 * E], w_in[2 * E :]
    bq, bk, bv = b_in[:E], b_in[E : 2 * E], b_in[2 * E :]
    scale = 1.0 / math.sqrt(DH)

    wq2 = (wq * ln1_g[None, :]) * scale
    bq2 = (wq @ ln1_b + bq) * scale
    wk2 = wk * ln1_g[None, :]
    bk2 = wk @ ln1_b + bk
    wv2 = wv * ln1_g[None, :]
    bv2 = wv @ ln1_b + bv
    bvec = b_out + w_out @ bv2
    wf2 = w_fc * ln2_g[None, :]
    bfc2 = b_fc + w_fc @ ln2_b
    bvec2 = b_proj

    def tile_w(wT2):
        # [out,in] weight -> SBUF layout [p, kc, o] with p the contraction row
        return np.ascontiguousarray(
            wT2.T.reshape(8, P, E).transpose(1, 0, 2).astype(bf)
        )

    wq_sb = tile_w(wq2)
    wk_sb = tile_w(wk2)
    wv_sb = tile_w(wv2)
    wo_sb = tile_w(w_out)
    # w_fc': [FF, E] -> [hg, p(k), hi, kc, c] so each hg slice is one DMA
    wf_sb = np.ascontiguousarray(
        wf2.reshape(8, 4, P, 8, P).transpose(0, 4, 1, 3, 2).astype(bf)
    )
    wpT = np.ascontiguousarray(w_proj.T)  # [FF, E]
    # [hg, p(h-row), hi, c]
    wpa_sb = np.ascontiguousarray(
        wpT[:, :512].reshape(8, 4, P, 512).transpose(0, 2, 1, 3).astype(bf)
    )
    wpb_sb = np.ascontiguousarray(
        wpT[:, 512:].reshape(8, 4, P, 512).transpose(0, 2, 1, 3).astype(bf)
    )
    bq_sb = np.ascontiguousarray(bq2.reshape(8, P).T)
    bk_sb = np.ascontiguousarray(bk2.reshape(8, P).T)
    bfc_sb = np.ascontiguousarray(bfc2.reshape(32, P).T)

    cm = np.maximum(causal_mask, NEG)

    in_maps = []
    rows_by_p = [_local_rows(0), _local_rows(1)]
    msk_by_p = []
    for p in range(2):
        msk = np.empty((P, 4, 256), np.float32)
        for t in range(4):
            ctx = S - 256 * t
            rows_t = rows_by_p[p][P * t : P * (t + 1)]
            msk[:, t, :] = cm[rows_t, ctx - 256 : ctx]
        msk_by_p.append(msk)

    shared = dict(
        wq_sb=wq_sb,
        wk_sb=wk_sb,
        wv_sb=wv_sb,
        wo_sb=wo_sb,
        wf_sb=wf_sb,
        wpa_sb=wpa_sb,
        wpb_sb=wpb_sb,
        bq_sb=bq_sb,
        bk_sb=bk_sb,
        bfc_sb=bfc_sb,
        bvec=np.ascontiguousarray(bvec),
        bvec2=np.ascontiguousarray(bvec2),
    )
    for c in range(NCORES):
        b, p = c // 2, c % 2
        m = dict(shared)
        xb = np.ascontiguousarray(x[b])
        xloc = np.ascontiguousarray(x[b][rows_by_p[p]])
        m["x_glob_bf"] = xb.astype(bf)
        m["x_loc"] = xloc
        m["x_loc_bf"] = xloc.astype(bf)
        m["msk"] = msk_by_p[p]
        in_maps.append(m)

    nc = _build()
    trace = bool(os.environ.get("KERNEL_TRACE"))
    res = bass_utils.run_bass_kernel_spmd(
        nc, in_maps, list(range(NCORES)), trace=trace
    )
    LAST_RESULTS = res

    x_out = np.empty((B, S, E), np.float32)
    att_w = np.empty((B, S, S), np.float32)
    for c in range(NCORES):
        b, p = c // 2, c % 2
        rows = rows_by_p[p]
        x_out[b][rows] = res.results[c]["y_out"]
        att_w[b][rows] = res.results[c]["aw_out"]
    return (x_out, att_w)


# revision 15
# speedup vs baseline: 1.5909x; 1.0037x over previous
"""Trainium2 Bass kernel for a pre-LN transformer block (B=4,S=1024,E=1024,H=16).

Sharding: 8 cores = (batch b, parity p). Core (b,p) computes the full block for
batch b restricted to query tokens {q : q % 2 == p} (512 tokens). K/V are
computed for all 1024 tokens of the batch (duplicated across the pair), which
makes every core's instruction stream identical (pure SPMD) with per-core
differences carried entirely by input data (x slices + boundary masks).

Local token order: tile t in 0..3 holds global queries 256*(3-t)+2j+p, so the
causal context per tile is 1024-256t and AV matmul N shrinks monotonically.
"""

import math
import os

import numpy as np

B, S, E, H = 4, 1024, 1024, 16
DH = E // H
FF = 4 * E
P = 128
NCORES = 8
LN_EPS = 1e-5
NEG = -30000.0

_BUILD_CACHE = {}
LAST_RESULTS = None


def _build():
    if "nc" in _BUILD_CACHE:
        return _BUILD_CACHE["nc"]

    import concourse.bacc as bacc
    import concourse.bass as bass
    import concourse.mybir as mybir
    import concourse.tile as tile

    f32 = mybir.dt.float32
    bf16 = mybir.dt.bfloat16
    AF = mybir.ActivationFunctionType
    ALU = mybir.AluOpType

    nc = bacc.Bacc(
        "TRN2",
        target_bir_lowering=False,
        debug=False,
        enable_asserts=False,
        num_devices=NCORES,
    )

    # ---- DRAM I/O ----
    xg_d = nc.dram_tensor("x_glob_bf", [S, E], bf16, kind="ExternalInput")
    xl_d = nc.dram_tensor("x_loc", [S // 2, E], f32, kind="ExternalInput")
    xlb_d = nc.dram_tensor("x_loc_bf", [S // 2, E], bf16, kind="ExternalInput")
    wq_d = nc.dram_tensor("wq_sb", [P, 8, E], bf16, kind="ExternalInput")
    wk_d = nc.dram_tensor("wk_sb", [P, 8, E], bf16, kind="ExternalInput")
    wv_d = nc.dram_tensor("wv_sb", [P, 8, E], bf16, kind="ExternalInput")
    wo_d = nc.dram_tensor("wo_sb", [P, 8, E], bf16, kind="ExternalInput")
    wf_d = nc.dram_tensor("wf_sb", [8, P, 4, 8, P], bf16, kind="ExternalInput")
    wpa_d = nc.dram_tensor("wpa_sb", [8, P, 4, 512], bf16, kind="ExternalInput")
    wpb_d = nc.dram_tensor("wpb_sb", [8, P, 4, 512], bf16, kind="ExternalInput")
    bq_d = nc.dram_tensor("bq_sb", [P, 8], f32, kind="ExternalInput")
    bk_d = nc.dram_tensor("bk_sb", [P, 8], f32, kind="ExternalInput")
    bfc_d = nc.dram_tensor("bfc_sb", [P, 32], f32, kind="ExternalInput")
    bvec_d = nc.dram_tensor("bvec", [E], f32, kind="ExternalInput")
    bvec2_d = nc.dram_tensor("bvec2", [E], f32, kind="ExternalInput")
    msk_d = nc.dram_tensor("msk", [P, 4, 256], f32, kind="ExternalInput")
    yout_d = nc.dram_tensor("y_out", [S // 2, E], f32, kind="ExternalOutput")
    awout_d = nc.dram_tensor("aw_out", [S // 2, E], f32, kind="ExternalOutput")

    xg = xg_d.ap()
    xl = xl_d.ap()
    xlb = xlb_d.ap()
    yout = yout_d.ap()
    awout = awout_d.ap()

    def bcast_ap(handle):
        ap = handle.ap()
        return bass.AP(tensor=ap.tensor, offset=ap.offset, ap=[[0, P]] + list(ap.ap))

    with tile.TileContext(nc) as tc:
        import contextlib

        stack = contextlib.ExitStack()
        with stack:
            persist = stack.enter_context(tc.tile_pool(name="persist", bufs=1))

            xres = persist.tile([P, 4, E], f32, tag="xres")
            ysb = persist.tile([P, 4, E], f32, tag="ysb")
            awsb = persist.tile([P, 4, S], f32, tag="awsb")
            awv = persist.tile([P, 4, S], bf16, tag="awv")
            z2T = persist.tile([P, 8, 512], bf16, tag="z2T")
            bvecb = persist.tile([P, E], f32, tag="bvecb")
            bvec2b = persist.tile([P, E], f32, tag="bvec2b")
            mskb = persist.tile([P, 4, 256], f32, tag="mskb")
            bqsb = persist.tile([P, 8], f32, tag="bqsb")
            bksb = persist.tile([P, 8], f32, tag="bksb")
            bfcsb = persist.tile([P, 32], f32, tag="bfcsb")
            epsb = persist.tile([P, 1], f32, tag="epsb")

            # constants
            nc.vector.memset(epsb[:], LN_EPS)
            nc.sync.dma_start(out=bqsb[:], in_=bq_d.ap())
            nc.sync.dma_start(out=bksb[:], in_=bk_d.ap())
            nc.sync.dma_start(out=bfcsb[:], in_=bfc_d.ap())
            nc.gpsimd.dma_start(out=mskb[:], in_=msk_d.ap())
            nc.gpsimd.dma_start(out=bvecb[:], in_=bcast_ap(bvec_d))
            nc.gpsimd.dma_start(out=bvec2b[:], in_=bcast_ap(bvec2_d))
            nc.vector.memset(awsb[:], 0.0)
            nc.vector.memset(awv[:], 0.0)

            # attention-scope persistent tensors (freed before MLP)
            import contextlib as _ctl
            astack = _ctl.ExitStack()
            apool = astack.enter_context(tc.tile_pool(name="attnbuf", bufs=1))
            hstack = _ctl.ExitStack()
            hpool = hstack.enter_context(tc.tile_pool(name="hbuf", bufs=1))
            hT = hpool.tile([P, 8, S], bf16, tag="hT")
            hqT = apool.tile([P, 8, 512], bf16, tag="hqT")
            kdt = apool.tile([P, 8, S], bf16, tag="kdt")
            vtd = apool.tile([P, 8, E], bf16, tag="vtd")
            qdt = apool.tile([P, 8, 512], bf16, tag="qdt")
            attsb = apool.tile([P, 8, 512], bf16, tag="attsb")

            # QKV weight pool opened early so transfers overlap LayerNorm
            wstack = _ctl.ExitStack()
            wp = wstack.enter_context(tc.tile_pool(name="wqkv", bufs=2))
            wq_sb = wp.tile([P, 8, E], bf16, tag="w", name="wq_sb")
            nc.scalar.dma_start(out=wq_sb[:], in_=wq_d.ap())
            wk_sb = wp.tile([P, 8, E], bf16, tag="w", name="wk_sb")
            nc.sync.dma_start(out=wk_sb[:], in_=wk_d.ap())

            # ---------- Phase A: LayerNorm1 (batched sweeps) + transpose ----------
            with tc.tile_pool(name="lnp", bufs=3) as lnp, tc.tile_pool(
                name="lnx", bufs=6
            ) as lnxp, tc.tile_pool(name="lns", bufs=2) as lns:

                _lnctr = [0]
                _dmae = [nc.gpsimd, nc.sync, nc.scalar]

                def ln_batch(srcs, dsts):
                    nb = len(srcs)
                    _lnctr[0] += 1
                    xts = []
                    mvb = lns.tile([P, nb, 2], f32, tag="ln_mv", name=f"mv{_lnctr[0]}")
                    for t in range(nb):
                        xt = lnxp.tile([P, E], bf16, tag="ln_x")
                        _dmae[t % 3].dma_start(out=xt[:], in_=srcs[t])
                        xts.append(xt)
                        st = lns.tile([P, 2, 6], f32, tag="ln_st", bufs=4)
                        nc.vector.bn_stats(out=st[:, 0, :], in_=xt[:, 0:512])
                        nc.vector.bn_stats(out=st[:, 1, :], in_=xt[:, 512:1024])
                        nc.vector.bn_aggr(out=mvb[:, t, :], in_=st[:])
                    sdb = lns.tile([P, nb], f32, tag="ln_sd")
                    nc.scalar.activation(
                        sdb[:], mvb[:, :, 1], AF.Sqrt, bias=epsb[:], scale=1.0
                    )
                    rsb = lns.tile([P, nb], f32, tag="ln_rs")
                    nc.vector.reciprocal(rsb[:], sdb[:])
                    nmub = lns.tile([P, nb], f32, tag="ln_nmu")
                    nc.vector.tensor_tensor(nmub[:], mvb[:, :, 0], rsb[:], op=ALU.mult)
                    nc.vector.tensor_scalar_mul(nmub[:], nmub[:], -1.0)
                    for t in range(nb):
                        zt = lnp.tile([P, E], bf16, tag="ln_z")
                        nc.scalar.activation(
                            zt[:],
                            xts[t][:],
                            AF.Identity,
                            bias=nmub[:, t : t + 1],
                            scale=rsb[:, t : t + 1],
                        )
                        nc.sync.dma_start_transpose(out=dsts[t], in_=zt[:])

                ln_batch(
                    [xlb[P * t : P * (t + 1), :] for t in range(4)],
                    [hqT[:, :, P * t : P * (t + 1)] for t in range(4)],
                )
                ln_batch(
                    [xg[P * t : P * (t + 1), :] for t in range(4)],
                    [hT[:, :, P * t : P * (t + 1)] for t in range(4)],
                )
                ln_batch(
                    [xg[P * t : P * (t + 1), :] for t in range(4, 8)],
                    [hT[:, :, P * t : P * (t + 1)] for t in range(4, 8)],
                )
                for m in range(4):
                    nc.gpsimd.dma_start(
                        out=xres[:, m, :], in_=xl[P * m : P * (m + 1), :]
                    )
                    nc.vector.tensor_add(xres[:, m, :], xres[:, m, :], bvecb[:])

            # ---------- Phase B: QKV projections ----------
            with tc.tile_pool(name="psqkv", bufs=4, space="PSUM") as pp:
                for oc in range(8):
                    ps = pp.tile([P, 512], f32, tag="ps")
                    for kc in range(8):
                        nc.tensor.matmul(
                            ps[:],
                            lhsT=wq_sb[:, kc, P * oc : P * (oc + 1)],
                            rhs=hqT[:, kc, :],
                            start=(kc == 0),
                            stop=(kc == 7),
                        )
                    nc.scalar.activation(
                        qdt[:, oc, :],
                        ps[:],
                        AF.Identity,
                        bias=bqsb[:, oc : oc + 1],
                        scale=1.0,
                    )
                for oc in range(8):
                    for ch in range(2):
                        ps = pp.tile([P, 512], f32, tag="ps")
                        for kc in range(8):
                            nc.tensor.matmul(
                                ps[:],
                                lhsT=wk_sb[:, kc, P * oc : P * (oc + 1)],
                                rhs=hT[:, kc, 512 * ch : 512 * (ch + 1)],
                                start=(kc == 0),
                                stop=(kc == 7),
                            )
                        nc.scalar.activation(
                            kdt[:, oc, 512 * ch : 512 * (ch + 1)],
                            ps[:],
                            AF.Identity,
                            bias=bksb[:, oc : oc + 1],
                            scale=1.0,
                        )
                wv_sb = wp.tile([P, 8, E], bf16, tag="w", name="wv_sb")
                nc.gpsimd.dma_start(out=wv_sb[:], in_=wv_d.ap())
                for tkt in range(8):
                    for ch in range(2):
                        ps = pp.tile([P, 512], f32, tag="ps")
                        for kc in range(8):
                            nc.tensor.matmul(
                                ps[:],
                                lhsT=hT[:, kc, P * tkt : P * (tkt + 1)],
                                rhs=wv_sb[:, kc, 512 * ch : 512 * (ch + 1)],
                                start=(kc == 0),
                                stop=(kc == 7),
                            )
                        nc.vector.tensor_copy(
                            vtd[:, tkt, 512 * ch : 512 * (ch + 1)], ps[:]
                        )

            wstack.close()
            hstack.close()

            # ---------- Phase C: attention ----------
            NK = [512, 512, 384, 384, 256, 256, 128, 128]
            with tc.tile_pool(name="wo", bufs=1) as wop:
                wo_sb = wop.tile([P, 8, E], bf16, tag="wo")
                nc.scalar.dma_start(out=wo_sb[:], in_=wo_d.ap())
                with tc.tile_pool(name="prb", bufs=2) as prp, tc.tile_pool(
                    name="psc", bufs=3, space="PSUM"
                ) as scp, tc.tile_pool(
                    name="psav", bufs=2, space="PSUM"
                ) as avp, tc.tile_pool(name="asm", bufs=8) as smp:

                    def emit_av(i, pTs):
                        avps = avp.tile([P, 512], f32, tag="av", name=f"av{i}")
                        for kc in range(8):
                            nk = NK[kc]
                            for sub in range(2):
                                h = 2 * i + sub
                                nc.tensor.matmul(
                                    avps[64 * sub : 64 * (sub + 1), :nk],
                                    lhsT=vtd[:, kc, 64 * h : 64 * (h + 1)],
                                    rhs=pTs[sub][:, kc, :nk],
                                    start=(kc == 0),
                                    stop=(kc == 7),
                                    skip_group_check=True,
                                )
                        nc.vector.tensor_copy(attsb[:, i, :], avps[:])

                    pend = None
                    for i in range(8):
                        pTs = [
                            prp.tile(
                                [P, 8, 512], bf16, tag="pT", name=f"pT{i}_{s}", bufs=4
                            )
                            for s in range(2)
                        ]
                        for tl in range(4):
                            ctx = S - 256 * tl
                            sc2 = [
                                scp.tile([P, S], f32, tag="sc", name=f"sc{i}_{tl}_{s}")
                                for s in range(2)
                            ]
                            for c0 in range(0, ctx, 512):
                                n = min(512, ctx - c0)
                                for sub in range(2):
                                    nc.tensor.matmul(
                                        sc2[sub][:, c0 : c0 + n],
                                        lhsT=qdt[
                                            64 * sub : 64 * (sub + 1),
                                            i,
                                            P * tl : P * (tl + 1),
                                        ],
                                        rhs=kdt[
                                            64 * sub : 64 * (sub + 1), i, c0 : c0 + n
                                        ],
                                        start=True,
                                        stop=True,
                                    )
                            for sub in range(2):
                                sc = sc2[sub]
                                nc.vector.tensor_add(
                                    sc[:, ctx - 256 : ctx],
                                    sc[:, ctx - 256 : ctx],
                                    mskb[:, tl, :],
                                )
                                pf = prp.tile([P, S], f32, tag="pf")
                                den = smp.tile([P, 1], f32, tag="den")
                                nc.scalar.activation(
                                    pf[:, :ctx],
                                    sc[:, :ctx],
                                    AF.Exp,
                                    accum_out=den[:],
                                )
                                r = smp.tile([P, 1], f32, tag="r")
                                nc.vector.reciprocal(r[:], den[:])
                                pbt = prp.tile([P, S], bf16, tag="pb", bufs=3)
                                if sub == 0:
                                    nc.vector.tensor_scalar(
                                        out=pbt[:, :ctx],
                                        in0=pf[:, :ctx],
                                        scalar1=r[:],
                                        scalar2=None,
                                        op0=ALU.mult,
                                    )
                                    nc.gpsimd.tensor_tensor(
                                        awsb[:, tl, :ctx],
                                        awsb[:, tl, :ctx],
                                        pbt[:, :ctx],
                                        op=ALU.add,
                                    )
                                else:
                                    nc.scalar.activation(
                                        pbt[:, :ctx],
                                        pf[:, :ctx],
                                        AF.Copy,
                                        scale=r[:],
                                    )
                                    nc.vector.tensor_tensor(
                                        awv[:, tl, :ctx],
                                        awv[:, tl, :ctx],
                                        pbt[:, :ctx],
                                        op=ALU.add,
                                    )
                                nc.sync.dma_start_transpose(
                                    out=pTs[sub][:, : ctx // P, P * tl : P * (tl + 1)],
                                    in_=pbt[:, :ctx],
                                )
                        if pend is not None:
                            emit_av(*pend)
                        pend = (i, pTs)
                    emit_av(*pend)

                # ---------- Phase D: out_proj + residual ----------
                with tc.tile_pool(name="pso", bufs=2, space="PSUM") as pop:
                    for m in range(4):
                        for o2 in range(2):
                            ps = pop.tile([P, 512], f32, tag="po")
                            for i in range(8):
                                nc.tensor.matmul(
                                    ps[:],
                                    lhsT=attsb[:, i, P * m : P * (m + 1)],
                                    rhs=wo_sb[:, i, 512 * o2 : 512 * (o2 + 1)],
                                    start=(i == 0),
                                    stop=(i == 7),
                                )
                            nc.vector.tensor_add(
                                ysb[:, m, 512 * o2 : 512 * (o2 + 1)],
                                ps[:],
                                xres[:, m, 512 * o2 : 512 * (o2 + 1)],
                            )

            # ---------- Phase E: LayerNorm2 + transpose ----------
            with tc.tile_pool(name="ln2p", bufs=3) as lnp2, tc.tile_pool(
                name="ln2s", bufs=4
            ) as lns2:
                for m in range(4):
                    st = lns2.tile([P, 2, 6], f32, tag="l2st")
                    nc.vector.bn_stats(out=st[:, 0, :], in_=ysb[:, m, 0:512])
                    nc.vector.bn_stats(out=st[:, 1, :], in_=ysb[:, m, 512:1024])
                    mv = lns2.tile([P, 2], f32, tag="l2mv")
                    nc.vector.bn_aggr(out=mv[:], in_=st[:])
                    sd = lns2.tile([P, 1], f32, tag="l2sd")
                    nc.scalar.activation(
                        sd[:], mv[:, 1:2], AF.Sqrt, bias=epsb[:], scale=1.0
                    )
                    rs = lns2.tile([P, 1], f32, tag="l2rs")
                    nc.vector.reciprocal(rs[:], sd[:])
                    nmu = lns2.tile([P, 1], f32, tag="l2nmu")
                    nc.vector.tensor_scalar(
                        out=nmu[:], in0=mv[:, 0:1], scalar1=rs[:], scalar2=-1.0,
                        op0=ALU.mult, op1=ALU.mult,
                    )
                    z2 = lnp2.tile([P, E], bf16, tag="l2z")
                    nc.scalar.activation(
                        z2[:], ysb[:, m, :], AF.Identity, bias=nmu[:], scale=rs[:]
                    )
                    nc.sync.dma_start_transpose(
                        out=z2T[:, :, P * m : P * (m + 1)], in_=z2[:]
                    )

            # close attention buffers before MLP
            astack.close()

            # ---------- Phase F/G: MLP ----------
            with tc.tile_pool(name="mlp", bufs=1) as mp, tc.tile_pool(
                name="blk", bufs=3
            ) as bp, tc.tile_pool(name="x3p", bufs=3) as xp, tc.tile_pool(
                name="pspj", bufs=1, space="PSUM"
            ) as jp:
                hidT = mp.tile([P, 32, 512], bf16, tag="hidT")
                psA = [
                    jp.tile([P, 512], f32, tag=f"pja{m}", name=f"pja{m}")
                    for m in range(4)
                ]
                wfap = wf_d.ap()
                wpaap = wpa_d.ap()
                wpbap = wpb_d.ap()
                with tc.tile_pool(name="psf", bufs=3, space="PSUM") as fp:
                    for hg in range(8):
                        bf4 = bp.tile([P, 4, 8, P], bf16, tag="bf4")
                        ([nc.scalar, nc.sync][hg % 2]).dma_start(out=bf4[:], in_=wfap[hg])
                        bpa4 = bp.tile([P, 4, 512], bf16, tag="bpa4")
                        nc.gpsimd.dma_start(out=bpa4[:], in_=wpaap[hg])
                        for hi in range(4):
                            hc = 4 * hg + hi
                            psF = fp.tile([P, 512], f32, tag="psF")
                            for kc in range(8):
                                nc.tensor.matmul(
                                    psF[:],
                                    lhsT=bf4[:, hi, kc, :],
                                    rhs=z2T[:, kc, :],
                                    start=(kc == 0),
                                    stop=(kc == 7),
                                )
                            nc.scalar.activation(
                                hidT[:, hc, :],
                                psF[:],
                                AF.Gelu_apprx_tanh,
                                bias=bfcsb[:, hc : hc + 1],
                                scale=1.0,
                            )
                            for m in range(4):
                                nc.tensor.matmul(
                                    psA[m][:],
                                    lhsT=hidT[:, hc, P * m : P * (m + 1)],
                                    rhs=bpa4[:, hi, :],
                                    start=(hc == 0),
                                    stop=(hc == 31),
                                )
                    for m in range(4):
                        x3 = xp.tile([P, 512], f32, tag="x3")
                        nc.vector.tensor_add(x3[:], psA[m][:], ysb[:, m, 0:512])
                        nc.vector.tensor_add(x3[:], x3[:], bvec2b[:, 0:512])
                        nc.gpsimd.dma_start(
                            out=yout[P * m : P * (m + 1), 0:512], in_=x3[:]
                        )
                # att_weights finalize: merge halves, scale by 1/H, store
                for tl in range(4):
                    nc.vector.tensor_tensor(
                        awsb[:, tl, :], awsb[:, tl, :], awv[:, tl, :], op=ALU.add
                    )
                    nc.vector.tensor_scalar_mul(
                        awsb[:, tl, :], awsb[:, tl, :], 1.0 / H
                    )
                    nc.gpsimd.dma_start(
                        out=awout[P * tl : P * (tl + 1), :], in_=awsb[:, tl, :]
                    )
                with tc.tile_pool(name="pspjb", bufs=1, space="PSUM") as jpb:
                    psB = [
                        jpb.tile([P, 512], f32, tag=f"pjb{m}", name=f"pjb{m}")
                        for m in range(4)
                    ]
                    for hg in range(8):
                        bpb4 = bp.tile([P, 4, 512], bf16, tag="bpb4")
                        ([nc.sync, nc.scalar][hg % 2]).dma_start(out=bpb4[:], in_=wpbap[hg])
                        for hi in range(4):
                            hc = 4 * hg + hi
                            for m in range(4):
                                nc.tensor.matmul(
                                    psB[m][:],
                                    lhsT=hidT[:, hc, P * m : P * (m + 1)],
                                    rhs=bpb4[:, hi, :],
                                    start=(hc == 0),
                                    stop=(hc == 31),
                                )
                    for m in range(4):
                        x3 = xp.tile([P, 512], f32, tag="x3")
                        nc.vector.tensor_add(x3[:], psB[m][:], ysb[:, m, 512:1024])
                        nc.vector.tensor_add(x3[:], x3[:], bvec2b[:, 512:1024])
                        nc.gpsimd.dma_start(
                            out=yout[P * m : P * (m + 1), 512:1024], in_=x3[:]
                        )

    nc.compile()
    _BUILD_CACHE["nc"] = nc
    return nc


def _local_rows(p):
    rows = []
    for t in range(4):
        g = 3 - t
        rows.extend(256 * g + 2 * j + p for j in range(P))
    return np.array(rows, dtype=np.int64)


def kernel(
    x,
    causal_mask,
    ln1_g,
    ln1_b,
    ln2_g,
    ln2_b,
    w_in,
    b_in,
    w_out,
    b_out,
    w_fc,
    b_fc,
    w_proj,
    b_proj,
):
    global LAST_RESULTS
    import ml_dtypes

    from concourse import bass_utils

    bf = ml_dtypes.bfloat16
    x = np.asarray(x, np.float32)
    causal_mask = np.asarray(causal_mask, np.float32)
    f32 = lambda a: np.ascontiguousarray(np.asarray(a, np.float32))
    ln1_g, ln1_b, ln2_g, ln2_b = map(f32, (ln1_g, ln1_b, ln2_g, ln2_b))
    w_in, b_in, w_out, b_out = map(f32, (w_in, b_in, w_out, b_out))
    w_fc, b_fc, w_proj, b_proj = map(f32, (w_fc, b_fc, w_proj, b_proj))

    wq, wk, wv = w_in[:E], w_in[E : 2 * E], w_in[2 * E :]
    bq, bk, bv = b_in[:E], b_in[E : 2 * E], b_in[2 * E :]
    scale = 1.0 / math.sqrt(DH)

    wq2 = (wq * ln1_g[None, :]) * scale
    bq2 = (wq @ ln1_b + bq) * scale
    wk2 = wk * ln1_g[None, :]
    bk2 = wk @ ln1_b + bk
    wv2 = wv * ln1_g[None, :]
    bv2 = wv @ ln1_b + bv
    bvec = b_out + w_out @ bv2
    wf2 = w_fc * ln2_g[None, :]
    bfc2 = b_fc + w_fc @ ln2_b
    bvec2 = b_proj

    def tile_w(wT2):
        # [out,in] weight -> SBUF layout [p, kc, o] with p the contraction row
        return np.ascontiguousarray(
            wT2.T.reshape(8, P, E).transpose(1, 0, 2).astype(bf)
        )

    wq_sb = tile_w(wq2)
    wk_sb = tile_w(wk2)
    wv_sb = tile_w(wv2)
    wo_sb = tile_w(w_out)
    # w_fc': [FF, E] -> [hg, p(k), hi, kc, c] so each hg slice is one DMA
    wf_sb = np.ascontiguousarray(
        wf2.reshape(8, 4, P, 8, P).transpose(0, 4, 1, 3, 2).astype(bf)
    )
    wpT = np.ascontiguousarray(w_proj.T)  # [FF, E]
    # [hg, p(h-row), hi, c]
    wpa_sb = np.ascontiguousarray(
        wpT[:, :512].reshape(8, 4, P, 512).transpose(0, 2, 1, 3).astype(bf)
    )
    wpb_sb = np.ascontiguousarray(
        wpT[:, 512:].reshape(8, 4, P, 512).transpose(0, 2, 1, 3).astype(bf)
    )
    bq_sb = np.ascontiguousarray(bq2.reshape(8, P).T)
    bk_sb = np.ascontiguousarray(bk2.reshape(8, P).T)
    bfc_sb = np.ascontiguousarray(bfc2.reshape(32, P).T)

    cm = np.maximum(causal_mask, NEG)

    in_maps = []
    rows_by_p = [_local_rows(0), _local_rows(1)]
    msk_by_p = []
    for p in range(2):
        msk = np.empty((P, 4, 256), np.float32)
        for t in range(4):
            ctx = S - 256 * t
            rows_t = rows_by_p[p][P * t : P * (t + 1)]
            msk[:, t, :] = cm[rows_t, ctx - 256 : ctx]
        msk_by_p.append(msk)

    shared = dict(
        wq_sb=wq_sb,
        wk_sb=wk_sb,
        wv_sb=wv_sb,
        wo_sb=wo_sb,
        wf_sb=wf_sb,
        wpa_sb=wpa_sb,
        wpb_sb=wpb_sb,
        bq_sb=bq_sb,
        bk_sb=bk_sb,
        bfc_sb=bfc_sb,
        bvec=np.ascontiguousarray(bvec),
        bvec2=np.ascontiguousarray(bvec2),
    )
    for c in range(NCORES):
        b, p = c // 2, c % 2
        m = dict(shared)
        xb = np.ascontiguousarray(x[b])
        xloc = np.ascontiguousarray(x[b][rows_by_p[p]])
        m["x_glob_bf"] = xb.astype(bf)
        m["x_loc"] = xloc
        m["x_loc_bf"] = xloc.astype(bf)
        m["msk"] = msk_by_p[p]
        in_maps.append(m)

    nc = _build()
    trace = bool(os.environ.get("KERNEL_TRACE"))
    res = bass_utils.run_bass_kernel_spmd(
        nc, in_maps, list(range(NCORES)), trace=trace
    )
    LAST_RESULTS = res

    x_out = np.empty((B, S, E), np.float32)
    att_w = np.empty((B, S, S), np.float32)
    for c in range(NCORES):
        b, p = c // 2, c % 2
        rows = rows_by_p[p]
        x_out[b][rows] = res.results[c]["y_out"]
        att_w[b][rows] = res.results[c]["aw_out"]
    return (x_out, att_w)
